# revision 1
# baseline (speedup 1.0000x reference)
"""GNN message-passing kernel for Trainium2 (8 NeuronCores, SPMD).

Strategy (hardcoded for the nn_DoormanAgent problem):
  - 65536 nodes = 64 graphs x 1024; shard 8192 nodes (8 graphs) per core.
  - Activations live transposed in SBUF: [128 HID partitions x nodes free], f32.
  - Per layer: u_loc = x @ W_aggr (node-major psum tiles) -> DRAM in two
    halves; two AllGathers produce tableA/tableB ([32768,128] bf16 each,
    int16-indexable).
  - Edges are grouped by (dst tile PAIR, src half) and packed contiguously
    into 128-slot chunks (no per-tile ceil padding); gpsimd.dma_gather
    fetches 256B rows; segment-sum on TensorE via one-hot S [slots, 256]
    built in bulk on VectorE (is_equal vs iota256, broadcast AP); psum is
    [128, 256] per tile pair; degree*b_aggr via a K=1 outer product.
  - A-half gathers run ahead with a lookahead window so desc-gen overlaps
    AllGather B; B gathers stream per batch.
  - Final BatchNorm via per-channel partial sums + a tiny AllReduce; last
    matmul 256->2 done per 128-node tile with a K=1 bias trick.
"""

import sys

sys.path.insert(0, "/opt/trn_rl_repo")

import numpy as np
import ml_dtypes

import concourse.bass as bass
import concourse.bacc as bacc
import concourse.mybir as mybir
import concourse.tile as tile
from concourse.bass_utils import run_bass_kernel_spmd
from concourse.library_config import mlp as mlp_library

BF16 = mybir.dt.bfloat16
F32 = mybir.dt.float32
I16 = mybir.dt.int16

N = 65536
E = 524288
NCORES = 8
NPC = N // NCORES            # 8192 nodes per core
TPC = NPC // 128             # 64 dst tiles per core
NPP = TPC // 2               # 32 dst tile-pairs per core (256 nodes each)
NPG = 1024                   # nodes per graph
GPC = NPC // NPG             # 8 graphs per core
IN_DIM, HID, OUT_DIM, LAYERS = 64, 128, 2, 3
EPS = 1e-5
HALFPC = NPC // 2            # 4096: rows per core per table half
TROWS = HALFPC * NCORES      # 32768 rows per table (int16-safe)
CALL_PAIRS = 2               # tile pairs per gather call (512 dst nodes)
NB = NPP // CALL_PAIRS       # 16 batches
LOOKA = 5                    # A-half gather lookahead (batches)

BULK_ST = True    # batched DVE one-hot builds

_cache = {}


def _host_prep(ei):
    """Group each core's incident edges by (dst tile pair, src half A/B),
    packed contiguously into 128-slot chunks; chunk counts are padded to the
    cross-core max so the SPMD instruction stream is identical on all
    cores."""
    src = np.asarray(ei[0], dtype=np.int64)
    dst = np.asarray(ei[1], dtype=np.int64)

    owner = src // NPC
    off = src % NPC
    tab_of = (off >= HALFPC).astype(np.int64)
    row_of = owner * HALFPC + off - tab_of * HALFPC
    core_of = dst // NPC
    pair_of = (dst % NPC) // 256
    rel_of = dst % 256

    counts = np.zeros((NCORES, NPP, 2), dtype=np.int64)
    groups = [[[None, None] for _ in range(NPP)] for _ in range(NCORES)]
    for c in range(NCORES):
        mc = core_of == c
        for h in (0, 1):
            m = mc & (tab_of == h)
            p_arr = pair_of[m]
            s_arr = row_of[m]
            r_arr = rel_of[m]
            order = np.argsort(p_arr, kind="stable")
            p_arr, s_arr, r_arr = p_arr[order], s_arr[order], r_arr[order]
            bounds = np.searchsorted(p_arr, np.arange(NPP + 1))
            for p in range(NPP):
                lo, hi = bounds[p], bounds[p + 1]
                groups[c][p][h] = (s_arr[lo:hi], r_arr[lo:hi])
                counts[c, p, h] = hi - lo

    gmax = counts.max(axis=0)                         # [NPP, 2]
    npk = -(-gmax // 128)                             # chunks per (pair, half)

    # call layout: per batch of CALL_PAIRS pairs, one call per half
    calls = []
    idx_cols = 0
    chunk_tot = 0
    for b in range(NB):
        prs = list(range(b * CALL_PAIRS, (b + 1) * CALL_PAIRS))
        for h in (0, 1):
            spans = []       # (pair, k, call-chunk pos)
            ck = 0
            for p in prs:
                for k in range(int(npk[p, h])):
                    spans.append((p, k, ck))
                    ck += 1
            calls.append(dict(half=h, spans=spans, nchunks=ck,
                              idx_col=idx_cols, chunk_off=chunk_tot))
            idx_cols += ck * 8
            chunk_tot += ck

    nkmax = max((c["nchunks"] for c in calls), default=1)
    sched = dict(npk=npk, calls=calls, idx_cols=idx_cols,
                 chunk_tot=chunk_tot, nkmax=nkmax)

    # per-core arrays
    per_core = []
    for c in range(NCORES):
        idx16 = np.zeros((16, max(idx_cols, 8)), dtype=np.int16)
        dstrel = np.full((128, max(chunk_tot, 1)), 999.0, dtype=np.float32)
        for call in calls:
            h = call["half"]
            for (p, k, ck) in call["spans"]:
                s_arr, r_arr = groups[c][p][h]
                sl = np.zeros(128, dtype=np.int16)
                rl = np.full(128, 999.0, dtype=np.float32)  # pad: no dst
                seg_s = s_arr[k * 128:(k + 1) * 128]
                seg_r = r_arr[k * 128:(k + 1) * 128]
                sl[:len(seg_s)] = seg_s.astype(np.int16)
                rl[:len(seg_r)] = -seg_r.astype(np.float32)  # negated rel256
                # slot i of chunk -> idx[(i % 16), base + i // 16]
                base = call["idx_col"] + ck * 8
                idx16[:, base:base + 8] = sl.reshape(8, 16).T
                dstrel[:, call["chunk_off"] + ck] = rl
        idx = np.tile(idx16, (8, 1))
        deg = np.bincount(dst[core_of == c] % NPC, minlength=NPC).astype(np.float32)
        per_core.append(dict(
            idx=idx,
            dstrel=dstrel,
            deg=deg.reshape(1, NPC).astype(ml_dtypes.bfloat16),
        ))
    return sched, per_core


def _build_nc(sched, nlayers=LAYERS):
    nc = bacc.Bacc("TRN2", target_bir_lowering=False, debug=False)

    # ---- dram parameters (inputs) ----
    p_xT0 = nc.declare_dram_parameter("xT0", [IN_DIM, NPC], F32, isOutput=False)
    p_idx = nc.declare_dram_parameter("idx", [128, max(sched["idx_cols"], 8)], I16, isOutput=False)
    p_dstrel = nc.declare_dram_parameter("dstrel", [128, max(sched["chunk_tot"], 1)], F32, isOutput=False)
    p_deg = nc.declare_dram_parameter("deg", [1, NPC], BF16, isOutput=False)
    p_iota = nc.declare_dram_parameter("iota", [128, 256], F32, isOutput=False)
    p_wproj = nc.declare_dram_parameter("wproj", [IN_DIM, HID], F32, isOutput=False)
    p_wl = nc.declare_dram_parameter("wl", [HID, LAYERS * HID], F32, isOutput=False)
    p_wa = nc.declare_dram_parameter("wa", [HID, LAYERS * HID], F32, isOutput=False)
    p_bag = nc.declare_dram_parameter("bag", [1, LAYERS * HID], BF16, isOutput=False)
    p_biaspb = nc.declare_dram_parameter("biaspb", [128, 1 + LAYERS], F32, isOutput=False)
    p_bn = nc.declare_dram_parameter("bn", [128, 4], F32, isOutput=False)
    p_wfx = nc.declare_dram_parameter("wfx", [HID, OUT_DIM], F32, isOutput=False)
    p_wfu = nc.declare_dram_parameter("wfu", [HID, OUT_DIM], F32, isOutput=False)
    p_bfin = nc.declare_dram_parameter("bfin", [GPC, OUT_DIM], F32, isOutput=False)
    p_ones = nc.declare_dram_parameter("ones1", [1, 128], F32, isOutput=False)
    p_out = nc.declare_dram_parameter("out", [NPC, OUT_DIM], F32, isOutput=True)

    AG_RG = [list(range(NCORES))]
    calls = sched["calls"]
    nkmax = sched["nkmax"]

    with tile.TileContext(nc) as tc:
        with (
            tc.tile_pool(name="const", bufs=1) as const,
            tc.tile_pool(name="acts", bufs=2) as acts,
            tc.tile_pool(name="gbpA", bufs=LOOKA + 1) as gbpA,
            tc.tile_pool(name="gbpB", bufs=2) as gbpB,
            tc.tile_pool(name="stp", bufs=2) as stp,
            tc.tile_pool(name="work", bufs=2) as work,
            tc.tile_pool(name="stats", bufs=1) as stats,
            tc.tile_pool(name="pscat", bufs=3, space="PSUM") as pscat,
            tc.tile_pool(name="pmisc", bufs=2, space="PSUM") as pmisc,
            tc.tile_pool(name="dram", bufs=2, space="DRAM") as dram,
        ):
            nc.gpsimd.load_library(mlp_library)

            # ---- load constants ----
            def cload(p, shape, dtype, tag):
                t = const.tile(shape, dtype, tag=tag, name=tag)
                nc.sync.dma_start(t[:], p[:, :])
                return t

            idx_sb = cload(p_idx, [128, max(sched["idx_cols"], 8)], I16, "idx")
            dstrel_sb = cload(p_dstrel, list(p_dstrel.shape), F32, "dstrel")
            deg_sb = cload(p_deg, [1, NPC], BF16, "deg")
            iota_sb = cload(p_iota, [128, 256], F32, "iota")
            wproj_sb = cload(p_wproj, [IN_DIM, HID], F32, "wproj")
            wl_sb = cload(p_wl, [HID, LAYERS * HID], F32, "wl")
            wa_sb = cload(p_wa, [HID, LAYERS * HID], F32, "wa")
            bag_sb = cload(p_bag, [1, LAYERS * HID], BF16, "bag")
            biaspb_sb = cload(p_biaspb, [128, 1 + LAYERS], F32, "biaspb")
            bn_sb = cload(p_bn, [128, 4], F32, "bn")
            wfx_sb = cload(p_wfx, [HID, OUT_DIM], F32, "wfx")
            wfu_sb = cload(p_wfu, [HID, OUT_DIM], F32, "wfu")
            bfin_sb = cload(p_bfin, [GPC, OUT_DIM], F32, "bfin")
            ones_sb = cload(p_ones, [1, 128], F32, "ones1")

            # iota256 replicated along a middle axis for bulk S-builds
            iota_rep = const.tile([128, nkmax, 256], F32, tag="iota_rep")
            for k in range(nkmax):
                nc.vector.tensor_copy(iota_rep[:, k, :], iota_sb[:])

            def new_uloc(h):
                return dram.tile([HALFPC, HID], BF16, tag=f"uloc{h}",
                                 name=f"uloc{h}")

            def emit_uloc_group(xsrc, wa_i, h, t4, uloc):
                """One [128, 512] group of u_loc half h (tiles 4*t4..4*t4+3
                within the half) -> DRAM."""
                u3 = uloc.rearrange("(t p) h -> t p h", p=128)
                ps = pmisc.tile([128, 512], F32, tag="mm512", name="ps")
                ub = work.tile([128, 512], BF16, tag="ubounce", name="ub")
                for q in range(4):
                    t = h * (TPC // 2) + 4 * t4 + q
                    nc.tensor.matmul(ps[:, q * 128:(q + 1) * 128],
                                     xsrc[:, t * 128:(t + 1) * 128], wa_i,
                                     start=True, stop=True)
                nc.scalar.activation(ub[:], ps[:],
                                     mybir.ActivationFunctionType.Copy)
                for q in range(4):
                    nc.sync.dma_start(u3[4 * t4 + q],
                                      ub[:, q * 128:(q + 1) * 128])

            def emit_ag(uloc, h):
                tbl = dram.tile([TROWS, HID], BF16, tag=f"table{h}",
                                name=f"table{h}")
                nc.gpsimd.collective_compute(
                    "AllGather", mybir.AluOpType.bypass,
                    replica_groups=AG_RG,
                    ins=[uloc[:].opt()],
                    outs=[tbl[:].opt()],
                )
                return tbl

            # ---- input projection + relu (x0 streamed in 512-col chunks);
            # layer-0 uloc groups interleave with proj; AG_A fires mid-proj,
            # AG_B is deferred into layer 0 (after its prefetch gathers)
            xT = acts.tile([HID, NPC], F32, tag="x")
            uloc_next = [new_uloc(0), new_uloc(1)]
            tableA_next = None
            for j in range(NPC // 512):
                x0c = work.tile([IN_DIM, 512], F32, tag="x0c")
                nc.sync.dma_start(x0c[:], p_xT0[:, j * 512:(j + 1) * 512])
                ps = pmisc.tile([128, 512], F32, tag="mm512", name="ps")
                nc.tensor.matmul(ps[:], wproj_sb[:], x0c[:],
                                 start=True, stop=True)
                nc.scalar.activation(xT[:, j * 512:(j + 1) * 512], ps[:],
                                     mybir.ActivationFunctionType.Relu,
                                     bias=biaspb_sb[:, 0:1])
                h, t4 = (0, j) if j < 8 else (1, j - 8)
                emit_uloc_group(xT, wa_sb[:, 0:HID], h, t4, uloc_next[h])
                if j == 7:
                    tableA_next = emit_ag(uloc_next[0], 0)
            ulocB_pending = uloc_next[1]

            ug_parts = stats.tile([128, NPP], F32, tag="ug_parts")
            sx_parts = stats.tile([128, NPP], F32, tag="sx_parts")
            ssx_parts = stats.tile([128, NPP], F32, tag="ssx_parts")
            scrap = stats.tile([128, 256], BF16, tag="scrap")

            # ---- message-passing layers ----
            for li in range(nlayers):
                wa_i = wa_sb[:, li * HID:(li + 1) * HID]
                wl_i = wl_sb[:, li * HID:(li + 1) * HID]
                bag_i = bag_sb[:, li * HID:(li + 1) * HID]
                wa_nx = wa_sb[:, (li + 1) * HID:(li + 2) * HID] if li + 1 < nlayers else None
                last = li == nlayers - 1

                tables = [tableA_next, None]
                tableA_next = None
                if not last:
                    uloc_next = [new_uloc(0), new_uloc(1)]

                xT_new = acts.tile([HID, NPC], F32, tag="x")

                def build_st(call):
                    # one-hot build: st[:, ck, j] = (dstrel[:, ck] == -j)
                    nck = call["nchunks"]
                    st = stp.tile([128, nck, 256], BF16,
                                  tag=f"st{call['half']}", name="st")
                    co = call["chunk_off"]
                    if BULK_ST:
                        # dstrel holds -d: compare (-iota) == (-d)
                        nc.vector.scalar_tensor_tensor(
                            st[:], iota_rep[:, 0:nck, :], -1.0,
                            dstrel_sb[:, co:co + nck, None].broadcast_to([128, nck, 256]),
                            mybir.AluOpType.mult, mybir.AluOpType.is_equal)
                    else:
                        # ScalarE: relu(1 - |iota - d|) is one-hot for ints
                        for ck in range(nck):
                            sttmp = work.tile([128, 256], BF16, tag="sttmp")
                            nc.scalar.activation(
                                sttmp[:], iota_sb[:],
                                mybir.ActivationFunctionType.Abs,
                                bias=dstrel_sb[:, co + ck:co + ck + 1])
                            nc.scalar.activation(
                                st[:, ck, :], sttmp[:],
                                mybir.ActivationFunctionType.Relu,
                                bias=1.0, scale=-1.0)
                    return st

                def gather(call, pool):
                    nck = call["nchunks"]
                    h = call["half"]
                    gb = pool.tile([128, nck, HID], BF16, tag=f"gb{h}",
                                   name="gb")
                    nidx = nck * 128
                    nc.gpsimd.dma_gather(
                        gb[:], tables[h][:, :],
                        idx_sb[:, call["idx_col"]:call["idx_col"] + nck * 8],
                        nidx, nidx, HID, single_packet=False,
                    )
                    return gb

                # A-half gathers run LOOKA batches ahead; the deferred AG_B
                # for THIS layer fires after them (its uloc stores are long
                # done, so the collective doesn't block the gpsimd queue, and
                # its latency hides under subsequent A gathers).
                gbA = {}
                for j in range(min(LOOKA, NB)):
                    gbA[j] = gather(calls[2 * j], gbpA)
                tables[1] = emit_ag(ulocB_pending, 1)

                for b in range(NB):
                    callA, callB = calls[2 * b], calls[2 * b + 1]
                    gbufs = {0: (gbA.pop(b), callA),
                             1: (gather(callB, gbpB), callB)}
                    sbufs = {0: build_st(callA), 1: build_st(callB)}

                    for p in range(b * CALL_PAIRS, (b + 1) * CALL_PAIRS):
                        pt = pscat.tile([128, 256], F32, tag="scat")
                        started = False
                        for h in (0, 1):
                            gb, call = gbufs[h]
                            st = sbufs[h]
                            for (pp, k, ck) in call["spans"]:
                                if pp != p:
                                    continue
                                nc.tensor.matmul(pt[:], gb[:, ck, :],
                                                 st[:, ck, :],
                                                 start=not started, stop=False)
                                started = True
                        # degree * b_aggr (completes u for this pair)
                        nc.tensor.matmul(pt[:], bag_i,
                                         deg_sb[:, p * 256:(p + 1) * 256],
                                         start=not started, stop=last)
                        if not last:
                            # x_i = x @ W_layers accumulated on top
                            nc.tensor.matmul(pt[:], wl_i,
                                             xT[:, p * 256:(p + 1) * 256],
                                             start=False, stop=True)
                            nc.scalar.activation(
                                xT_new[:, p * 256:(p + 1) * 256], pt[:],
                                mybir.ActivationFunctionType.Relu,
                                bias=biaspb_sb[:, li + 1:li + 2])
                        else:
                            # u finished: per-pair u sums on ScalarE, then add
                            # x_i from a separate psum tile on DVE + relu.
                            nc.scalar.activation(
                                scrap[:], pt[:],
                                mybir.ActivationFunctionType.Copy,
                                accum_out=ug_parts[:, p:p + 1])
                            pxi = pmisc.tile([128, 256], F32, tag="mmfin")
                            nc.tensor.matmul(pxi[:], wl_i,
                                             xT[:, p * 256:(p + 1) * 256],
                                             start=True, stop=True)
                            xi_sb = work.tile([128, 256], F32, tag="xisb")
                            nc.scalar.activation(
                                xi_sb[:], pxi[:],
                                mybir.ActivationFunctionType.Copy)
                            tmp = work.tile([128, 256], F32, tag="xtmp")
                            nc.vector.scalar_tensor_tensor(
                                tmp[:], pt[:], biaspb_sb[:, li + 1:li + 2],
                                xi_sb[:], mybir.AluOpType.add,
                                mybir.AluOpType.add)
                            nc.vector.tensor_scalar(
                                xT_new[:, p * 256:(p + 1) * 256], tmp[:], 0.0,
                                None, mybir.AluOpType.max)
                            # BatchNorm sums inline (ScalarE)
                            nc.scalar.activation(
                                scrap[:], xT_new[:, p * 256:(p + 1) * 256],
                                mybir.ActivationFunctionType.Copy,
                                accum_out=sx_parts[:, p:p + 1])
                            nc.scalar.activation(
                                scrap[:], xT_new[:, p * 256:(p + 1) * 256],
                                mybir.ActivationFunctionType.Square,
                                accum_out=ssx_parts[:, p:p + 1])
                    nxt = b + LOOKA
                    if nxt < NB:
                        gbA[nxt] = gather(calls[2 * nxt], gbpA)
                    # next layer's uloc groups stream per batch as xT_new
                    # pairs complete; AG_A fires two batches after its half's
                    # stores so the collective never blocks the gpsimd queue
                    if not last:
                        if b < NB // 2:
                            emit_uloc_group(xT_new, wa_nx, 0, b, uloc_next[0])
                        else:
                            emit_uloc_group(xT_new, wa_nx, 1, b - NB // 2,
                                            uloc_next[1])
                        if b == NB // 2 + 1:
                            tableA_next = emit_ag(uloc_next[0], 0)
                if not last:
                    ulocB_pending = uloc_next[1]
                xT = xT_new

            # ---- BatchNorm statistics (sums already accumulated inline) ----
            ug = stats.tile([128, GPC], F32, tag="ug")
            for g in range(GPC):
                nc.vector.tensor_reduce(ug[:, g:g + 1],
                                        ug_parts[:, g * 4:(g + 1) * 4],
                                        mybir.AxisListType.X, mybir.AluOpType.add)
            ugsq = stats.tile([128, GPC], F32, tag="ugsq")
            nc.vector.scalar_tensor_tensor(ugsq[:], ug[:], 0.0, ug[:],
                                           mybir.AluOpType.bypass,
                                           mybir.AluOpType.mult)
            pack = stats.tile([128, 4], F32, tag="pack")
            nc.vector.tensor_reduce(pack[:, 0:1], sx_parts[:],
                                    mybir.AxisListType.X, mybir.AluOpType.add)
            nc.vector.tensor_reduce(pack[:, 1:2], ssx_parts[:],
                                    mybir.AxisListType.X, mybir.AluOpType.add)
            nc.vector.tensor_reduce(pack[:, 2:3], ug[:],
                                    mybir.AxisListType.X, mybir.AluOpType.add)
            nc.vector.tensor_reduce(pack[:, 3:4], ugsq[:],
                                    mybir.AxisListType.X, mybir.AluOpType.add)
            # scale u-channel partials by nodes-per-graph
            nc.vector.tensor_scalar_mul(pack[:, 2:3], pack[:, 2:3], float(NPG))
            nc.vector.tensor_scalar_mul(pack[:, 3:4], pack[:, 3:4], float(NPG))

            ar_in = dram.tile([128, 4], F32, tag="ar_in")
            ar_out = dram.tile([128, 4], F32, tag="ar_out")
            nc.sync.dma_start(ar_in[:], pack[:])
            nc.gpsimd.collective_compute(
                "AllReduce", mybir.AluOpType.add,
                replica_groups=AG_RG,
                ins=[ar_in[:].opt()],
                outs=[ar_out[:].opt()],
            )
            gstats = stats.tile([128, 4], F32, tag="gstats")
            nc.sync.dma_start(gstats[:], ar_out[:])

            # mean/var -> scale/bias per channel, for x-half and u-half
            sb = {}
            for half_i, (s_col, q_col, g_col, b_col) in enumerate(
                    [(0, 1, 0, 1), (2, 3, 2, 3)]):
                mean = stats.tile([128, 1], F32, tag=f"mean{half_i}")
                var = stats.tile([128, 1], F32, tag=f"var{half_i}")
                rstd = stats.tile([128, 1], F32, tag=f"rstd{half_i}")
                scl = stats.tile([128, 1], F32, tag=f"scl{half_i}")
                bia = stats.tile([128, 1], F32, tag=f"bia{half_i}")
                nc.vector.tensor_scalar_mul(mean[:], gstats[:, s_col:s_col + 1], 1.0 / N)
                nc.vector.tensor_scalar_mul(var[:], gstats[:, q_col:q_col + 1], 1.0 / N)
                tmp = stats.tile([128, 1], F32, tag=f"tmp{half_i}")
                nc.vector.scalar_tensor_tensor(tmp[:], mean[:], 0.0, mean[:],
                                               mybir.AluOpType.bypass,
                                               mybir.AluOpType.mult)
                nc.vector.scalar_tensor_tensor(var[:], var[:], 0.0, tmp[:],
                                               mybir.AluOpType.bypass,
                                               mybir.AluOpType.subtract)
                std = stats.tile([128, 1], F32, tag=f"std{half_i}")
                nc.vector.tensor_scalar_add(var[:], var[:], EPS)
                nc.scalar.activation(std[:], var[:],
                                     mybir.ActivationFunctionType.Sqrt)
                nc.vector.reciprocal(rstd[:], std[:])
                nc.vector.scalar_tensor_tensor(scl[:], rstd[:], 0.0,
                                               bn_sb[:, g_col:g_col + 1],
                                               mybir.AluOpType.bypass,
                                               mybir.AluOpType.mult)
                nc.vector.scalar_tensor_tensor(tmp[:], mean[:], 0.0, scl[:],
                                               mybir.AluOpType.bypass,
                                               mybir.AluOpType.mult)
                nc.vector.scalar_tensor_tensor(bia[:], bn_sb[:, b_col:b_col + 1],
                                               0.0, tmp[:],
                                               mybir.AluOpType.bypass,
                                               mybir.AluOpType.subtract)
                sb[half_i] = (scl, bia)

            # fold BN into the final matmul: out = xT @ (scl_x*wfx)
            #   + [ug @ (scl_u*wfu) + b_final + sum_ch(bia*W)] per graph
            wfxs = stats.tile([128, OUT_DIM], F32, tag="wfxs")
            nc.vector.tensor_scalar(wfxs[:], wfx_sb[:], sb[0][0][:], None,
                                    mybir.AluOpType.mult)
            wfus = stats.tile([128, OUT_DIM], F32, tag="wfus")
            nc.vector.tensor_scalar(wfus[:], wfu_sb[:], sb[1][0][:], None,
                                    mybir.AluOpType.mult)
            # bterm[1,2] = sum_ch bia_x*wfx + bia_u*wfu
            pb = pmisc.tile([1, OUT_DIM], F32, tag="mmfin")
            nc.tensor.matmul(pb[:], sb[0][1][:], wfx_sb[:], start=True, stop=False)
            nc.tensor.matmul(pb[:], sb[1][1][:], wfu_sb[:], start=False, stop=True)
            pb_sb = stats.tile([1, OUT_DIM], F32, tag="pb_sb")
            nc.scalar.activation(pb_sb[:], pb[:],
                                 mybir.ActivationFunctionType.Copy)

            # c_u[g,:] = ug[:,g] @ wfus + bterm (broadcast via K=1 matmul)
            cu_ps = pmisc.tile([GPC, OUT_DIM], F32, tag="mmfin")
            nc.tensor.matmul(cu_ps[:], ug[:], wfus[:], start=True, stop=False)
            nc.tensor.matmul(cu_ps[:], ones_sb[:, 0:GPC], pb_sb[:],
                             start=False, stop=True)
            cub = stats.tile([GPC, OUT_DIM], F32, tag="cub")
            nc.vector.scalar_tensor_tensor(cub[:], cu_ps[:], 0.0, bfin_sb[:],
                                           mybir.AluOpType.bypass,
                                           mybir.AluOpType.add)
            cub_dram = dram.tile([GPC, OUT_DIM], F32, tag="cub_dram")
            nc.sync.dma_start(cub_dram[:], cub[:])
            cubrow = stats.tile([1, GPC * OUT_DIM], F32, tag="cubrow")
            nc.sync.dma_start(
                cubrow[:], cub_dram[:].rearrange("g o -> (g o)")[None, :])

            # final matmul per tile + bias via K=1 trick (raw xT, scaled W);
            # results staged in SBUF, written out with a single DMA
            out_sb = stats.tile([128, TPC, OUT_DIM], F32, tag="out_sb")
            for t in range(TPC):
                g = t // 8
                psf = pmisc.tile([128, OUT_DIM], F32, tag="mmfin")
                nc.tensor.matmul(psf[:], xT[:, t * 128:(t + 1) * 128], wfxs[:],
                                 start=True, stop=False)
                nc.tensor.matmul(psf[:], ones_sb[:],
                                 cubrow[:, g * OUT_DIM:(g + 1) * OUT_DIM],
                                 start=False, stop=True)
                nc.vector.tensor_copy(out_sb[:, t, :], psf[:])
            nc.sync.dma_start(
                p_out[:, :].rearrange("(t p) o -> p t o", p=128), out_sb[:])

    nc.compile()
    return nc


def _bf16(a):
    return np.asarray(a, dtype=np.float32).astype(ml_dtypes.bfloat16)


def _make_in_maps(per_core, x, W_proj, b_proj, W_layers, b_layers, W_aggr,
                  b_aggr, bn_gamma, bn_beta, W_final, b_final):
    x = np.asarray(x, dtype=np.float32)
    iota_t = np.tile(np.arange(256, dtype=np.float32), (128, 1))
    shared = dict(
        iota=iota_t.astype(np.float32),
        wproj=np.asarray(W_proj, np.float32),
        wl=np.concatenate(list(np.asarray(W_layers, np.float32)), axis=1),
        wa=np.concatenate(list(np.asarray(W_aggr, np.float32)), axis=1),
        bag=_bf16(np.asarray(b_aggr, np.float32).reshape(1, LAYERS * HID)),
        biaspb=np.concatenate(
            [np.asarray(b_proj, np.float32).reshape(128, 1),
             np.asarray(b_layers, np.float32).T], axis=1).astype(np.float32),
        bn=np.stack([np.asarray(bn_gamma, np.float32)[:128],
                     np.asarray(bn_beta, np.float32)[:128],
                     np.asarray(bn_gamma, np.float32)[128:],
                     np.asarray(bn_beta, np.float32)[128:]], axis=1).astype(np.float32),
        wfx=np.asarray(W_final, np.float32)[:HID],
        wfu=np.asarray(W_final, np.float32)[HID:],
        bfin=np.tile(np.asarray(b_final, np.float32).reshape(1, OUT_DIM),
                     (GPC, 1)).astype(np.float32),
        ones1=np.ones((1, 128), np.float32),
    )
    in_maps = []
    for c in range(NCORES):
        m = dict(shared)
        m["xT0"] = np.ascontiguousarray(x[c * NPC:(c + 1) * NPC].T)
        m["idx"] = per_core[c]["idx"]
        m["dstrel"] = per_core[c]["dstrel"]
        m["deg"] = per_core[c]["deg"]
        in_maps.append(m)
    return in_maps


def kernel(x, ei, n_nodes, W_proj, b_proj, W_layers, b_layers, W_aggr, b_aggr,
           bn_gamma, bn_beta, W_final, b_final):
    key = hash(np.asarray(ei).tobytes())
    if key not in _cache:
        sched, per_core = _host_prep(ei)
        nc = _build_nc(sched)
        _cache[key] = (nc, per_core)
    nc, per_core = _cache[key]
    in_maps = _make_in_maps(per_core, x, W_proj, b_proj, W_layers, b_layers,
                            W_aggr, b_aggr, bn_gamma, bn_beta, W_final, b_final)
    global _last_in_maps
    _last_in_maps = in_maps
    res = run_bass_kernel_spmd(nc, in_maps, core_ids=list(range(NCORES)))
    out = np.concatenate([res.results[c]["out"] for c in range(NCORES)], axis=0)
    return out.reshape(N // int(n_nodes), -1).astype(np.float32)


_last_in_maps = None



# revision 5
# speedup vs baseline: 1.3138x; 1.3138x over previous
"""GNN message-passing kernel for Trainium2 (8 NeuronCores, SPMD). v2

Strategy (hardcoded for the nn_DoormanAgent problem):
  - 65536 nodes = 64 graphs x 1024; shard 8192 nodes (8 graphs) per core.
  - Activations live transposed in SBUF: [128 HID partitions x nodes free], f32.
  - Per layer: u_loc = x @ W_aggr + b_aggr (bias folded into the table) ->
    DRAM in two halves; two AllGathers produce tableA/tableB
    ([32768,128] bf16 each, int16-indexable) into Shared scratchpad.
  - Edges grouped per call = (dst tile PAIR x 2, src half): slot stream is
    [p0 edges | gap pad(idx 0) | p1 edges | trailing -1]; trailing -1 idxs
    are skipped by the Q7 desc-gen (cost ~= actual edges, not padded).
    The boundary chunk is "mixed" and is matmul'd once per pair with
    separate one-hot columns.
  - gpsimd.dma_gather desc-gen is the bottleneck engine; calls rotate over
    4 SWDGE queues so desc-gen parallelizes across Q7 core pairs (~2.8x).
  - Segment-sum on TensorE via one-hot S [slots, 256] built in bulk on
    VectorE in bf16 (2x DVE rate); psum is [128, 256] per dst pair.
  - A-half gathers run ahead with a lookahead window; AG_B for layer i
    fires at the END of layer i-1 so B gathers never stall.
  - Final BatchNorm via per-channel partial sums + a tiny AllReduce; last
    matmul 256->2 done per 128-node tile with a K=1 bias trick.
"""

import sys

sys.path.insert(0, "/opt/trn_rl_repo")

import numpy as np
import ml_dtypes

import concourse.bass as bass
import concourse.bacc as bacc
import concourse.mybir as mybir
import concourse.tile as tile
from concourse.bass_utils import run_bass_kernel_spmd
from concourse.library_config import mlp as mlp_library

BF16 = mybir.dt.bfloat16
F32 = mybir.dt.float32
I16 = mybir.dt.int16

N = 65536
E = 524288
NCORES = 8
NPC = N // NCORES            # 8192 nodes per core
TPC = NPC // 128             # 64 dst tiles per core
NPP = TPC // 2               # 32 dst tile-pairs per core (256 nodes each)
NPG = 1024                   # nodes per graph
GPC = NPC // NPG             # 8 graphs per core
IN_DIM, HID, OUT_DIM, LAYERS = 64, 128, 2, 3
EPS = 1e-5
HALFPC = NPC // 2            # 4096: rows per core per table half
TROWS = HALFPC * NCORES      # 32768 rows per table (int16-safe)
NB = NPP                     # 32... overwritten below
CALL_PAIRS = 2               # tile pairs per gather call (512 dst nodes)
NB = NPP // CALL_PAIRS       # 16 batches
LOOKA = 5                    # A-half gather lookahead (batches)
NQUEUES = 4                  # SWDGE queues (desc-gen core-pair parallelism)

_cache = {}


def _host_prep(ei):
    """Group each core's incident edges per call = (dst tile pair, src half).

    Call slot stream: [p0 edges | gap pad (idx 0) | p1 edges | trailing -1].
    Trailing -1 indices are skipped by Q7 desc-gen.  The chunk straddling
    the p0/p1 boundary ("mixed") gets two one-hot columns.  Chunk counts
    are cross-core maxima so the SPMD instruction stream is identical."""
    src = np.asarray(ei[0], dtype=np.int64)
    dst = np.asarray(ei[1], dtype=np.int64)

    owner = src // NPC
    off = src % NPC
    tab_of = (off >= HALFPC).astype(np.int64)
    row_of = owner * HALFPC + off - tab_of * HALFPC
    core_of = dst // NPC
    pair_of = (dst % NPC) // 256
    rel_of = dst % 256

    # groups[c][p][h] = (rows, rels)
    groups = [[[None, None] for _ in range(NPP)] for _ in range(NCORES)]
    for c in range(NCORES):
        mc = core_of == c
        for h in (0, 1):
            m = mc & (tab_of == h)
            p_arr = pair_of[m]
            s_arr = row_of[m]
            r_arr = rel_of[m]
            order = np.argsort(p_arr, kind="stable")
            p_arr, s_arr, r_arr = p_arr[order], s_arr[order], r_arr[order]
            bounds = np.searchsorted(p_arr, np.arange(NPP + 1))
            for p in range(NPP):
                lo, hi = bounds[p], bounds[p + 1]
                groups[c][p][h] = (s_arr[lo:hi], r_arr[lo:hi])

    calls = []
    idx_cols = 0
    st_cols = 0
    for b in range(NB):
        p0, p1 = CALL_PAIRS * b, CALL_PAIRS * b + 1
        for h in (0, 1):
            n0 = [len(groups[c][p0][h][0]) for c in range(NCORES)]
            n1 = [len(groups[c][p1][h][0]) for c in range(NCORES)]
            K0 = max(1, max(-(-n // 128) for n in n0))
            ends = [max(n0[c], (K0 - 1) * 128) + n1[c] for c in range(NCORES)]
            nck = max(K0, max(-(-e // 128) for e in ends))
            spans0 = [(k, k) for k in range(K0)]
            spans1 = [(K0 - 1, K0)] + [(k, k + 1) for k in range(K0, nck)]
            calls.append(dict(half=h, p0=p0, p1=p1, K0=K0, nck=nck,
                              idx_col=idx_cols, st_off=st_cols,
                              spans0=spans0, spans1=spans1))
            idx_cols += nck * 8
            st_cols += nck + 1

    nckmax = max(c["nck"] for c in calls)
    sched = dict(calls=calls, idx_cols=idx_cols, st_cols=st_cols,
                 nckmax=nckmax)

    per_core = []
    for c in range(NCORES):
        idx16 = np.zeros((16, max(idx_cols, 8)), dtype=np.int16)
        dstrel = np.full((128, max(st_cols, 1)), 1000.0, dtype=np.float32)
        for call in calls:
            h, p0, p1, K0, nck = (call["half"], call["p0"], call["p1"],
                                  call["K0"], call["nck"])
            s0, r0 = groups[c][p0][h]
            s1, r1 = groups[c][p1][h]
            n0, n1 = len(s0), len(s1)
            p1s = max(n0, (K0 - 1) * 128)
            # all pads fetch row 0 (codes 1000 -> ignored); trailing -1 would
            # desync the sequencer-side ring bookkeeping from the Q7 desc-gen
            stream = np.zeros(nck * 128, dtype=np.int16)
            stream[:n0] = s0.astype(np.int16)
            stream[p1s:p1s + n1] = s1.astype(np.int16)
            # wrap: stream pos j -> idx16[j % 16, idx_col + j // 16]
            base = call["idx_col"]
            idx16[:, base:base + nck * 8] = stream.reshape(nck * 8, 16).T
            # one-hot codes, negated; 1000 = no dst
            so = call["st_off"]
            codes = np.full((128, nck + 1), 1000.0, dtype=np.float32)
            for j, r in enumerate(r0):          # p0 edges: cols 0..K0-1
                codes[j % 128, j // 128] = -float(r)
            for j2, r in enumerate(r1):         # p1 edges
                j = p1s + j2
                ck = j // 128
                col = K0 if ck == K0 - 1 else ck + 1
                codes[j % 128, col] = -float(r)
            dstrel[:, so:so + nck + 1] = codes
        idx = np.tile(idx16, (8, 1))
        per_core.append(dict(
            idx=idx,
            dstrel=dstrel.astype(ml_dtypes.bfloat16),
        ))
    return sched, per_core


def _build_nc(sched, nlayers=LAYERS):
    nc = bacc.Bacc("TRN2", target_bir_lowering=False, debug=False,
                   num_swdge_queues=NQUEUES)

    # ---- dram parameters (inputs) ----
    p_xT0 = nc.declare_dram_parameter("xT0", [IN_DIM, NPC], F32, isOutput=False)
    p_idx = nc.declare_dram_parameter("idx", [128, max(sched["idx_cols"], 8)], I16, isOutput=False)
    p_dstrel = nc.declare_dram_parameter("dstrel", [128, max(sched["st_cols"], 1)], BF16, isOutput=False)
    p_iota = nc.declare_dram_parameter("iota", [128, 256], BF16, isOutput=False)
    p_wproj = nc.declare_dram_parameter("wproj", [IN_DIM, HID], F32, isOutput=False)
    p_wl = nc.declare_dram_parameter("wl", [HID, LAYERS * HID], F32, isOutput=False)
    p_wa = nc.declare_dram_parameter("wa", [HID, LAYERS * HID], F32, isOutput=False)
    p_bag4 = nc.declare_dram_parameter("bag4", [1, LAYERS * 512], F32, isOutput=False)
    p_biaspb = nc.declare_dram_parameter("biaspb", [128, 1 + LAYERS], F32, isOutput=False)
    p_bn = nc.declare_dram_parameter("bn", [128, 4], F32, isOutput=False)
    p_wfx = nc.declare_dram_parameter("wfx", [HID, OUT_DIM], F32, isOutput=False)
    p_wfu = nc.declare_dram_parameter("wfu", [HID, OUT_DIM], F32, isOutput=False)
    p_bfin = nc.declare_dram_parameter("bfin", [GPC, OUT_DIM], F32, isOutput=False)
    p_ones = nc.declare_dram_parameter("ones1", [1, 128], F32, isOutput=False)
    p_out = nc.declare_dram_parameter("out", [NPC, OUT_DIM], F32, isOutput=True)

    AG_RG = [list(range(NCORES))]
    calls = sched["calls"]
    NCK = sched["nckmax"]

    qctr = [0]

    with tile.TileContext(nc) as tc:
        with (
            tc.tile_pool(name="const", bufs=1) as const,
            tc.tile_pool(name="acts", bufs=2) as acts,
            tc.tile_pool(name="gbpA", bufs=LOOKA + 1) as gbpA,
            tc.tile_pool(name="gbpB", bufs=2) as gbpB,
            tc.tile_pool(name="stp", bufs=2) as stp,
            tc.tile_pool(name="work", bufs=2) as work,
            tc.tile_pool(name="stats", bufs=1) as stats,
            tc.tile_pool(name="pscat", bufs=3, space="PSUM") as pscat,
            tc.tile_pool(name="pmisc", bufs=2, space="PSUM") as pmisc,
            tc.tile_pool(name="dram", bufs=2, space="DRAM") as dram,
        ):
            nc.gpsimd.load_library(mlp_library)

            # ---- load constants ----
            def cload(p, shape, dtype, tag):
                t = const.tile(shape, dtype, tag=tag, name=tag)
                nc.sync.dma_start(t[:], p[:, :])
                return t

            idx_sb = cload(p_idx, [128, max(sched["idx_cols"], 8)], I16, "idx")
            dstrel_sb = cload(p_dstrel, list(p_dstrel.shape), BF16, "dstrel")
            iota_sb = cload(p_iota, [128, 256], BF16, "iota")
            wproj_sb = cload(p_wproj, [IN_DIM, HID], F32, "wproj")
            wl_sb = cload(p_wl, [HID, LAYERS * HID], F32, "wl")
            wa_sb = cload(p_wa, [HID, LAYERS * HID], F32, "wa")
            bag4_sb = cload(p_bag4, [1, LAYERS * 512], F32, "bag4")
            biaspb_sb = cload(p_biaspb, [128, 1 + LAYERS], F32, "biaspb")
            bn_sb = cload(p_bn, [128, 4], F32, "bn")
            wfx_sb = cload(p_wfx, [HID, OUT_DIM], F32, "wfx")
            wfu_sb = cload(p_wfu, [HID, OUT_DIM], F32, "wfu")
            bfin_sb = cload(p_bfin, [GPC, OUT_DIM], F32, "bfin")
            ones_sb = cload(p_ones, [1, 128], F32, "ones1")

            # iota256 replicated along the chunk axis for bulk S-builds
            iota_rep = const.tile([128, NCK + 1, 256], BF16, tag="iota_rep")
            for k in range(NCK + 1):
                nc.vector.tensor_copy(iota_rep[:, k, :], iota_sb[:])

            # zero the gather buffers once: trailing -1 slots are skipped by
            # desc-gen, so those partitions keep stale SBUF data (NaN shield).
            for _ in range(LOOKA + 1):
                t = gbpA.tile([128, NCK, HID], BF16, tag="gb0", name="gbz")
                nc.vector.memset(t[:], 0.0)
            for _ in range(2):
                t = gbpB.tile([128, NCK, HID], BF16, tag="gb1", name="gbz")
                nc.vector.memset(t[:], 0.0)

            def new_uloc(h):
                return dram.tile([HALFPC, HID], BF16, tag=f"uloc{h}",
                                 name=f"uloc{h}")

            def emit_uloc_group(xsrc, wa_i, li_target, h, t4, uloc):
                """One [128, 512] group of u_loc half h (tiles 4*t4..4*t4+3
                within the half) -> DRAM, with b_aggr folded in."""
                u3 = uloc.rearrange("(t p) h -> t p h", p=128)
                ps = pmisc.tile([128, 512], F32, tag="mm512", name="ps")
                ub = work.tile([128, 512], BF16, tag="ubounce", name="ub")
                # b_aggr folded in via K=1 ones-outer-product matmuls
                for q in range(4):
                    t = h * (TPC // 2) + 4 * t4 + q
                    co = li_target * 512 + q * 128
                    nc.tensor.matmul(ps[:, q * 128:(q + 1) * 128],
                                     ones_sb[:], bag4_sb[:, co:co + 128],
                                     start=True, stop=False)
                    nc.tensor.matmul(ps[:, q * 128:(q + 1) * 128],
                                     xsrc[:, t * 128:(t + 1) * 128], wa_i,
                                     start=False, stop=True)
                nc.scalar.activation(ub[:], ps[:],
                                     mybir.ActivationFunctionType.Copy)
                for q in range(4):
                    nc.sync.dma_start(u3[4 * t4 + q],
                                      ub[:, q * 128:(q + 1) * 128])

            def emit_ag(uloc, h, par):
                tbl = dram.tile([TROWS, HID], BF16, tag=f"table{h}",
                                name=f"table{h}")
                nc.gpsimd.collective_compute(
                    "AllGather", mybir.AluOpType.bypass,
                    replica_groups=AG_RG,
                    ins=[uloc[:].opt()],
                    outs=[tbl[:].opt()],
                )
                return tbl

            # ---- input projection + relu (x0 streamed in 512-col chunks);
            # layer-0 uloc groups interleave with proj; AG_A fires mid-proj,
            # AG_B right after proj (both tables ready before layer 0).
            xT = acts.tile([HID, NPC], F32, tag="x")
            uloc_next = [new_uloc(0), new_uloc(1)]
            tables_next = [None, None]
            for j in range(NPC // 512):
                x0c = work.tile([IN_DIM, 512], F32, tag="x0c")
                nc.sync.dma_start(x0c[:], p_xT0[:, j * 512:(j + 1) * 512])
                ps = pmisc.tile([128, 512], F32, tag="mm512", name="ps")
                nc.tensor.matmul(ps[:], wproj_sb[:], x0c[:],
                                 start=True, stop=True)
                nc.scalar.activation(xT[:, j * 512:(j + 1) * 512], ps[:],
                                     mybir.ActivationFunctionType.Relu,
                                     bias=biaspb_sb[:, 0:1])
                h, t4 = (0, j) if j < 8 else (1, j - 8)
                emit_uloc_group(xT, wa_sb[:, 0:HID], 0, h, t4, uloc_next[h])
                if j == 7:
                    tables_next[0] = emit_ag(uloc_next[0], 0, 0)
            tables_next[1] = emit_ag(uloc_next[1], 1, 0)

            ug_parts = stats.tile([128, NPP], F32, tag="ug_parts")
            sx_parts = stats.tile([128, NPP], F32, tag="sx_parts")
            ssx_parts = stats.tile([128, NPP], F32, tag="ssx_parts")
            scrap = stats.tile([128, 256], BF16, tag="scrap")

            # ---- message-passing layers ----
            for li in range(nlayers):
                wl_i = wl_sb[:, li * HID:(li + 1) * HID]
                wa_nx = wa_sb[:, (li + 1) * HID:(li + 2) * HID] if li + 1 < nlayers else None
                last = li == nlayers - 1

                tables = tables_next
                tables_next = [None, None]
                if not last:
                    uloc_next = [new_uloc(0), new_uloc(1)]

                xT_new = acts.tile([HID, NPC], F32, tag="x")

                def build_st(call):
                    # one-hot build: st[:, col, j] = (dstrel[:, col] == -j)
                    ncols = call["nck"] + 1
                    st = stp.tile([128, NCK + 1, 256], BF16,
                                  tag=f"st{call['half']}", name="st")
                    so = call["st_off"]
                    nc.vector.scalar_tensor_tensor(
                        st[:, 0:ncols, :], iota_rep[:, 0:ncols, :], -1.0,
                        dstrel_sb[:, so:so + ncols, None].broadcast_to([128, ncols, 256]),
                        mybir.AluOpType.mult, mybir.AluOpType.is_equal)
                    return st

                def gather(call, pool):
                    nck = call["nck"]
                    h = call["half"]
                    gb = pool.tile([128, NCK, HID], BF16, tag=f"gb{h}",
                                   name="gb")
                    nidx = nck * 128
                    nc.gpsimd.dma_gather(
                        gb[:, 0:nck, :], tables[h][:, :],
                        idx_sb[:, call["idx_col"]:call["idx_col"] + nck * 8],
                        nidx, nidx, HID, single_packet=False,
                        queue_num=qctr[0] % NQUEUES,
                    )
                    qctr[0] += 1
                    return gb

                # A-half gathers run LOOKA batches ahead.
                gbA = {}
                for j in range(min(LOOKA, NB)):
                    gbA[j] = gather(calls[2 * j], gbpA)

                for b in range(NB):
                    callA, callB = calls[2 * b], calls[2 * b + 1]
                    gbufs = {0: (gbA.pop(b), callA),
                             1: (gather(callB, gbpB), callB)}
                    sbufs = {0: build_st(callA), 1: build_st(callB)}

                    for p in (callA["p0"], callA["p1"]):
                        pt = pscat.tile([128, 256], F32, tag="scat")
                        # collect spans over both halves
                        todo = []
                        for h in (0, 1):
                            gb, call = gbufs[h]
                            st = sbufs[h]
                            spans = call["spans0"] if p == call["p0"] else call["spans1"]
                            for (ck, sc) in spans:
                                todo.append((gb, st, ck, sc))
                        for i, (gb, st, ck, sc) in enumerate(todo):
                            fin = last and i == len(todo) - 1
                            nc.tensor.matmul(pt[:], gb[:, ck, :],
                                             st[:, sc, :],
                                             start=(i == 0), stop=fin)
                        if not last:
                            # x_i = x @ W_layers accumulated on top
                            nc.tensor.matmul(pt[:], wl_i,
                                             xT[:, p * 256:(p + 1) * 256],
                                             start=False, stop=True)
                            nc.scalar.activation(
                                xT_new[:, p * 256:(p + 1) * 256], pt[:],
                                mybir.ActivationFunctionType.Relu,
                                bias=biaspb_sb[:, li + 1:li + 2])
                        else:
                            # u finished: per-pair u sums on ScalarE, then add
                            # x_i from a separate psum tile on DVE + relu.
                            nc.scalar.activation(
                                scrap[:], pt[:],
                                mybir.ActivationFunctionType.Copy,
                                accum_out=ug_parts[:, p:p + 1])
                            pxi = pmisc.tile([128, 256], F32, tag="mmfin")
                            nc.tensor.matmul(pxi[:], wl_i,
                                             xT[:, p * 256:(p + 1) * 256],
                                             start=True, stop=True)
                            xi_sb = work.tile([128, 256], F32, tag="xisb")
                            nc.scalar.activation(
                                xi_sb[:], pxi[:],
                                mybir.ActivationFunctionType.Copy)
                            tmp = work.tile([128, 256], F32, tag="xtmp")
                            nc.vector.scalar_tensor_tensor(
                                tmp[:], pt[:], biaspb_sb[:, li + 1:li + 2],
                                xi_sb[:], mybir.AluOpType.add,
                                mybir.AluOpType.add)
                            nc.vector.tensor_scalar(
                                xT_new[:, p * 256:(p + 1) * 256], tmp[:], 0.0,
                                None, mybir.AluOpType.max)
                            # BatchNorm sums inline (ScalarE)
                            nc.scalar.activation(
                                scrap[:], xT_new[:, p * 256:(p + 1) * 256],
                                mybir.ActivationFunctionType.Copy,
                                accum_out=sx_parts[:, p:p + 1])
                            nc.scalar.activation(
                                scrap[:], xT_new[:, p * 256:(p + 1) * 256],
                                mybir.ActivationFunctionType.Square,
                                accum_out=ssx_parts[:, p:p + 1])
                    nxt = b + LOOKA
                    if nxt < NB:
                        gbA[nxt] = gather(calls[2 * nxt], gbpA)
                    # next layer's uloc groups stream per batch as xT_new
                    # pairs complete; AG_A fires two batches after its half's
                    # stores complete so it is ready well before layer li+1.
                    if not last:
                        if b < NB // 2:
                            emit_uloc_group(xT_new, wa_nx, li + 1, 0, b,
                                            uloc_next[0])
                        else:
                            emit_uloc_group(xT_new, wa_nx, li + 1, 1,
                                            b - NB // 2, uloc_next[1])
                        if b == NB // 2 + 1:
                            tables_next[0] = emit_ag(uloc_next[0], 0,
                                                     (li + 1) % 2)
                # AG_B for the next layer fires at the end of this layer so
                # its latency hides under the next layer's A gathers.
                if not last:
                    tables_next[1] = emit_ag(uloc_next[1], 1, (li + 1) % 2)
                xT = xT_new

            # ---- BatchNorm statistics (sums already accumulated inline) ----
            ug = stats.tile([128, GPC], F32, tag="ug")
            for g in range(GPC):
                nc.vector.tensor_reduce(ug[:, g:g + 1],
                                        ug_parts[:, g * 4:(g + 1) * 4],
                                        mybir.AxisListType.X, mybir.AluOpType.add)
            ugsq = stats.tile([128, GPC], F32, tag="ugsq")
            nc.vector.scalar_tensor_tensor(ugsq[:], ug[:], 0.0, ug[:],
                                           mybir.AluOpType.bypass,
                                           mybir.AluOpType.mult)
            pack = stats.tile([128, 4], F32, tag="pack")
            nc.vector.tensor_reduce(pack[:, 0:1], sx_parts[:],
                                    mybir.AxisListType.X, mybir.AluOpType.add)
            nc.vector.tensor_reduce(pack[:, 1:2], ssx_parts[:],
                                    mybir.AxisListType.X, mybir.AluOpType.add)
            nc.vector.tensor_reduce(pack[:, 2:3], ug[:],
                                    mybir.AxisListType.X, mybir.AluOpType.add)
            nc.vector.tensor_reduce(pack[:, 3:4], ugsq[:],
                                    mybir.AxisListType.X, mybir.AluOpType.add)
            # scale u-channel partials by nodes-per-graph
            nc.vector.tensor_scalar_mul(pack[:, 2:3], pack[:, 2:3], float(NPG))
            nc.vector.tensor_scalar_mul(pack[:, 3:4], pack[:, 3:4], float(NPG))

            ar_in = dram.tile([128, 4], F32, tag="ar_in")
            ar_out = dram.tile([128, 4], F32, tag="ar_out")
            nc.sync.dma_start(ar_in[:], pack[:])
            nc.gpsimd.collective_compute(
                "AllReduce", mybir.AluOpType.add,
                replica_groups=AG_RG,
                ins=[ar_in[:].opt()],
                outs=[ar_out[:].opt()],
            )
            gstats = stats.tile([128, 4], F32, tag="gstats")
            nc.sync.dma_start(gstats[:], ar_out[:])

            # mean/var -> scale/bias per channel, for x-half and u-half
            sb = {}
            for half_i, (s_col, q_col, g_col, b_col) in enumerate(
                    [(0, 1, 0, 1), (2, 3, 2, 3)]):
                mean = stats.tile([128, 1], F32, tag=f"mean{half_i}")
                var = stats.tile([128, 1], F32, tag=f"var{half_i}")
                rstd = stats.tile([128, 1], F32, tag=f"rstd{half_i}")
                scl = stats.tile([128, 1], F32, tag=f"scl{half_i}")
                bia = stats.tile([128, 1], F32, tag=f"bia{half_i}")
                nc.vector.tensor_scalar_mul(mean[:], gstats[:, s_col:s_col + 1], 1.0 / N)
                nc.vector.tensor_scalar_mul(var[:], gstats[:, q_col:q_col + 1], 1.0 / N)
                tmp = stats.tile([128, 1], F32, tag=f"tmp{half_i}")
                nc.vector.scalar_tensor_tensor(tmp[:], mean[:], 0.0, mean[:],
                                               mybir.AluOpType.bypass,
                                               mybir.AluOpType.mult)
                nc.vector.scalar_tensor_tensor(var[:], var[:], 0.0, tmp[:],
                                               mybir.AluOpType.bypass,
                                               mybir.AluOpType.subtract)
                std = stats.tile([128, 1], F32, tag=f"std{half_i}")
                nc.vector.tensor_scalar_add(var[:], var[:], EPS)
                nc.scalar.activation(std[:], var[:],
                                     mybir.ActivationFunctionType.Sqrt)
                nc.vector.reciprocal(rstd[:], std[:])
                nc.vector.scalar_tensor_tensor(scl[:], rstd[:], 0.0,
                                               bn_sb[:, g_col:g_col + 1],
                                               mybir.AluOpType.bypass,
                                               mybir.AluOpType.mult)
                nc.vector.scalar_tensor_tensor(tmp[:], mean[:], 0.0, scl[:],
                                               mybir.AluOpType.bypass,
                                               mybir.AluOpType.mult)
                nc.vector.scalar_tensor_tensor(bia[:], bn_sb[:, b_col:b_col + 1],
                                               0.0, tmp[:],
                                               mybir.AluOpType.bypass,
                                               mybir.AluOpType.subtract)
                sb[half_i] = (scl, bia)

            # fold BN into the final matmul: out = xT @ (scl_x*wfx)
            #   + [ug @ (scl_u*wfu) + b_final + sum_ch(bia*W)] per graph
            wfxs = stats.tile([128, OUT_DIM], F32, tag="wfxs")
            nc.vector.tensor_scalar(wfxs[:], wfx_sb[:], sb[0][0][:], None,
                                    mybir.AluOpType.mult)
            wfus = stats.tile([128, OUT_DIM], F32, tag="wfus")
            nc.vector.tensor_scalar(wfus[:], wfu_sb[:], sb[1][0][:], None,
                                    mybir.AluOpType.mult)
            # bterm[1,2] = sum_ch bia_x*wfx + bia_u*wfu
            pb = pmisc.tile([1, OUT_DIM], F32, tag="mmfin")
            nc.tensor.matmul(pb[:], sb[0][1][:], wfx_sb[:], start=True, stop=False)
            nc.tensor.matmul(pb[:], sb[1][1][:], wfu_sb[:], start=False, stop=True)
            pb_sb = stats.tile([1, OUT_DIM], F32, tag="pb_sb")
            nc.scalar.activation(pb_sb[:], pb[:],
                                 mybir.ActivationFunctionType.Copy)

            # c_u[g,:] = ug[:,g] @ wfus + bterm (broadcast via K=1 matmul)
            cu_ps = pmisc.tile([GPC, OUT_DIM], F32, tag="mmfin")
            nc.tensor.matmul(cu_ps[:], ug[:], wfus[:], start=True, stop=False)
            nc.tensor.matmul(cu_ps[:], ones_sb[:, 0:GPC], pb_sb[:],
                             start=False, stop=True)
            cub = stats.tile([GPC, OUT_DIM], F32, tag="cub")
            nc.vector.scalar_tensor_tensor(cub[:], cu_ps[:], 0.0, bfin_sb[:],
                                           mybir.AluOpType.bypass,
                                           mybir.AluOpType.add)
            cub_dram = dram.tile([GPC, OUT_DIM], F32, tag="cub_dram")
            nc.sync.dma_start(cub_dram[:], cub[:])
            cubrow = stats.tile([1, GPC * OUT_DIM], F32, tag="cubrow")
            nc.sync.dma_start(
                cubrow[:], cub_dram[:].rearrange("g o -> (g o)")[None, :])

            # final matmul per tile + bias via K=1 trick (raw xT, scaled W);
            # results staged in SBUF, written out with a single DMA
            out_sb = stats.tile([128, TPC, OUT_DIM], F32, tag="out_sb")
            for t in range(TPC):
                g = t // 8
                psf = pmisc.tile([128, OUT_DIM], F32, tag="mmfin")
                nc.tensor.matmul(psf[:], xT[:, t * 128:(t + 1) * 128], wfxs[:],
                                 start=True, stop=False)
                nc.tensor.matmul(psf[:], ones_sb[:],
                                 cubrow[:, g * OUT_DIM:(g + 1) * OUT_DIM],
                                 start=False, stop=True)
                nc.vector.tensor_copy(out_sb[:, t, :], psf[:])
            nc.sync.dma_start(
                p_out[:, :].rearrange("(t p) o -> p t o", p=128), out_sb[:])

    nc.compile()
    return nc


def _bf16(a):
    return np.asarray(a, dtype=np.float32).astype(ml_dtypes.bfloat16)


def _make_in_maps(per_core, x, W_proj, b_proj, W_layers, b_layers, W_aggr,
                  b_aggr, bn_gamma, bn_beta, W_final, b_final):
    x = np.asarray(x, dtype=np.float32)
    iota_t = np.tile(np.arange(256, dtype=np.float32), (128, 1))
    shared = dict(
        iota=_bf16(iota_t),
        wproj=np.asarray(W_proj, np.float32),
        wl=np.concatenate(list(np.asarray(W_layers, np.float32)), axis=1),
        wa=np.concatenate(list(np.asarray(W_aggr, np.float32)), axis=1),
        bag4=np.concatenate([np.tile(np.asarray(b_aggr, np.float32)[li], 4)
                             for li in range(LAYERS)]).reshape(1, LAYERS * 512),
        biaspb=np.concatenate(
            [np.asarray(b_proj, np.float32).reshape(128, 1),
             np.asarray(b_layers, np.float32).T], axis=1).astype(np.float32),
        bn=np.stack([np.asarray(bn_gamma, np.float32)[:128],
                     np.asarray(bn_beta, np.float32)[:128],
                     np.asarray(bn_gamma, np.float32)[128:],
                     np.asarray(bn_beta, np.float32)[128:]], axis=1).astype(np.float32),
        wfx=np.asarray(W_final, np.float32)[:HID],
        wfu=np.asarray(W_final, np.float32)[HID:],
        bfin=np.tile(np.asarray(b_final, np.float32).reshape(1, OUT_DIM),
                     (GPC, 1)).astype(np.float32),
        ones1=np.ones((1, 128), np.float32),
    )
    in_maps = []
    for c in range(NCORES):
        m = dict(shared)
        m["xT0"] = np.ascontiguousarray(x[c * NPC:(c + 1) * NPC].T)
        m["idx"] = per_core[c]["idx"]
        m["dstrel"] = per_core[c]["dstrel"]
        in_maps.append(m)
    return in_maps


def kernel(x, ei, n_nodes, W_proj, b_proj, W_layers, b_layers, W_aggr, b_aggr,
           bn_gamma, bn_beta, W_final, b_final):
    key = hash(np.asarray(ei).tobytes())
    if key not in _cache:
        sched, per_core = _host_prep(ei)
        nc = _build_nc(sched)
        _cache[key] = (nc, per_core)
    nc, per_core = _cache[key]
    in_maps = _make_in_maps(per_core, x, W_proj, b_proj, W_layers, b_layers,
                            W_aggr, b_aggr, bn_gamma, bn_beta, W_final, b_final)
    global _last_in_maps
    _last_in_maps = in_maps
    res = run_bass_kernel_spmd(nc, in_maps, core_ids=list(range(NCORES)))
    out = np.concatenate([res.results[c]["out"] for c in range(NCORES)], axis=0)
    return out.reshape(N // int(n_nodes), -1).astype(np.float32)


_last_in_maps = None


# revision 8
# speedup vs baseline: 1.4214x; 1.0820x over previous
"""GNN message-passing kernel for Trainium2 (8 NeuronCores, SPMD). v2

Strategy (hardcoded for the nn_DoormanAgent problem):
  - 65536 nodes = 64 graphs x 1024; shard 8192 nodes (8 graphs) per core.
  - Activations live transposed in SBUF: [128 HID partitions x nodes free], f32.
  - Per layer: u_loc = x @ W_aggr + b_aggr (bias folded into the table) ->
    DRAM in two halves; two AllGathers produce tableA/tableB
    ([32768,128] bf16 each, int16-indexable) into Shared scratchpad.
  - Edges grouped per call = (dst tile PAIR x 2, src half): slot stream is
    [p0 edges | gap pad(idx 0) | p1 edges | trailing -1]; trailing -1 idxs
    are skipped by the Q7 desc-gen (cost ~= actual edges, not padded).
    The boundary chunk is "mixed" and is matmul'd once per pair with
    separate one-hot columns.
  - gpsimd.dma_gather desc-gen is the bottleneck engine; calls rotate over
    4 SWDGE queues so desc-gen parallelizes across Q7 core pairs (~2.8x).
  - Segment-sum on TensorE via one-hot S [slots, 256] built in bulk on
    VectorE in bf16 (2x DVE rate); psum is [128, 256] per dst pair.
  - A-half gathers run ahead with a lookahead window; AG_B for layer i
    fires at the END of layer i-1 so B gathers never stall.
  - Final BatchNorm via per-channel partial sums + a tiny AllReduce; last
    matmul 256->2 done per 128-node tile with a K=1 bias trick.
"""

import sys

sys.path.insert(0, "/opt/trn_rl_repo")

import numpy as np
import ml_dtypes

import concourse.bass as bass
import concourse.bacc as bacc
import concourse.mybir as mybir
import concourse.tile as tile
from concourse.bass_utils import run_bass_kernel_spmd
from concourse.library_config import mlp as mlp_library

BF16 = mybir.dt.bfloat16
F32 = mybir.dt.float32
I16 = mybir.dt.int16

N = 65536
E = 524288
NCORES = 8
NPC = N // NCORES            # 8192 nodes per core
TPC = NPC // 128             # 64 dst tiles per core
NPP = TPC // 2               # 32 dst tile-pairs per core (256 nodes each)
NPG = 1024                   # nodes per graph
GPC = NPC // NPG             # 8 graphs per core
IN_DIM, HID, OUT_DIM, LAYERS = 64, 128, 2, 3
EPS = 1e-5
HALFPC = NPC // 2            # 4096: rows per core per table half
TROWS = HALFPC * NCORES      # 32768 rows per table (int16-safe)
NB = NPP                     # 32... overwritten below
CALL_PAIRS = 2               # tile pairs per gather call (512 dst nodes)
NB = NPP // CALL_PAIRS       # 16 batches
LOOKA = 7                    # A-half gather lookahead (batches)
NQUEUES = 4                  # SWDGE queues (desc-gen core-pair parallelism)

_cache = {}


def _host_prep(ei):
    """Group each core's incident edges per call = (dst tile pair, src half).

    Call slot stream: [p0 edges | gap pad (idx 0) | p1 edges | trailing -1].
    Trailing -1 indices are skipped by Q7 desc-gen.  The chunk straddling
    the p0/p1 boundary ("mixed") gets two one-hot columns.  Chunk counts
    are cross-core maxima so the SPMD instruction stream is identical."""
    src = np.asarray(ei[0], dtype=np.int64)
    dst = np.asarray(ei[1], dtype=np.int64)

    owner = src // NPC
    off = src % NPC
    tab_of = (off >= HALFPC).astype(np.int64)
    row_of = owner * HALFPC + off - tab_of * HALFPC
    core_of = dst // NPC
    pair_of = (dst % NPC) // 256
    rel_of = dst % 256

    # groups[c][p][h] = (rows, rels)
    groups = [[[None, None] for _ in range(NPP)] for _ in range(NCORES)]
    for c in range(NCORES):
        mc = core_of == c
        for h in (0, 1):
            m = mc & (tab_of == h)
            p_arr = pair_of[m]
            s_arr = row_of[m]
            r_arr = rel_of[m]
            order = np.argsort(p_arr, kind="stable")
            p_arr, s_arr, r_arr = p_arr[order], s_arr[order], r_arr[order]
            bounds = np.searchsorted(p_arr, np.arange(NPP + 1))
            for p in range(NPP):
                lo, hi = bounds[p], bounds[p + 1]
                groups[c][p][h] = (s_arr[lo:hi], r_arr[lo:hi])

    calls = []
    idx_cols = 0
    st_cols = 0
    for b in range(NB):
        p0, p1 = CALL_PAIRS * b, CALL_PAIRS * b + 1
        for h in (0, 1):
            n0 = [len(groups[c][p0][h][0]) for c in range(NCORES)]
            n1 = [len(groups[c][p1][h][0]) for c in range(NCORES)]
            K0 = max(1, max(-(-n // 128) for n in n0))
            ends = [max(n0[c], (K0 - 1) * 128) + n1[c] for c in range(NCORES)]
            nck = max(K0, max(-(-e // 128) for e in ends))
            spans0 = [(k, k) for k in range(K0)]
            spans1 = [(K0 - 1, K0)] + [(k, k + 1) for k in range(K0, nck)]
            calls.append(dict(half=h, p0=p0, p1=p1, K0=K0, nck=nck,
                              idx_col=idx_cols, st_off=st_cols,
                              spans0=spans0, spans1=spans1))
            idx_cols += nck * 8
            st_cols += nck + 1

    nckmax = max(c["nck"] for c in calls)
    sched = dict(calls=calls, idx_cols=idx_cols, st_cols=st_cols,
                 nckmax=nckmax)

    per_core = []
    for c in range(NCORES):
        idx16 = np.zeros((16, max(idx_cols, 8)), dtype=np.int16)
        dstrel = np.full((128, max(st_cols, 1)), 1000.0, dtype=np.float32)
        for call in calls:
            h, p0, p1, K0, nck = (call["half"], call["p0"], call["p1"],
                                  call["K0"], call["nck"])
            s0, r0 = groups[c][p0][h]
            s1, r1 = groups[c][p1][h]
            n0, n1 = len(s0), len(s1)
            p1s = max(n0, (K0 - 1) * 128)
            stream = np.zeros(nck * 128, dtype=np.int16)
            stream[:n0] = s0.astype(np.int16)
            stream[p1s:p1s + n1] = s1.astype(np.int16)
            # wrap: stream pos j -> idx16[j % 16, idx_col + j // 16]
            base = call["idx_col"]
            idx16[:, base:base + nck * 8] = stream.reshape(nck * 8, 16).T
            # one-hot codes, negated; 1000 = no dst
            so = call["st_off"]
            codes = np.full((128, nck + 1), 1000.0, dtype=np.float32)
            for j, r in enumerate(r0):          # p0 edges: cols 0..K0-1
                codes[j % 128, j // 128] = -float(r)
            for j2, r in enumerate(r1):         # p1 edges
                j = p1s + j2
                ck = j // 128
                col = K0 if ck == K0 - 1 else ck + 1
                codes[j % 128, col] = -float(r)
            dstrel[:, so:so + nck + 1] = codes
        idx = np.tile(idx16, (8, 1))
        per_core.append(dict(
            idx=idx,
            dstrel=dstrel.astype(ml_dtypes.bfloat16),
        ))
    return sched, per_core


def _build_nc(sched, nlayers=LAYERS):
    nc = bacc.Bacc("TRN2", target_bir_lowering=False, debug=False,
                   num_swdge_queues=NQUEUES)

    # ---- dram parameters (inputs) ----
    p_xT0 = nc.declare_dram_parameter("xT0", [IN_DIM, NPC], F32, isOutput=False)
    p_idx = nc.declare_dram_parameter("idx", [128, max(sched["idx_cols"], 8)], I16, isOutput=False)
    p_dstrel = nc.declare_dram_parameter("dstrel", [128, max(sched["st_cols"], 1)], BF16, isOutput=False)
    p_iota = nc.declare_dram_parameter("iota", [128, 256], BF16, isOutput=False)
    p_wproj = nc.declare_dram_parameter("wproj", [IN_DIM, HID], F32, isOutput=False)
    p_wl = nc.declare_dram_parameter("wl", [HID, LAYERS * HID], F32, isOutput=False)
    p_wa = nc.declare_dram_parameter("wa", [HID, LAYERS * HID], F32, isOutput=False)
    p_bag4 = nc.declare_dram_parameter("bag4", [1, LAYERS * 512], F32, isOutput=False)
    p_biaspb = nc.declare_dram_parameter("biaspb", [128, 1 + LAYERS], F32, isOutput=False)
    p_bn = nc.declare_dram_parameter("bn", [128, 4], F32, isOutput=False)
    p_wfx = nc.declare_dram_parameter("wfx", [HID, OUT_DIM], F32, isOutput=False)
    p_wfu = nc.declare_dram_parameter("wfu", [HID, OUT_DIM], F32, isOutput=False)
    p_bfin = nc.declare_dram_parameter("bfin", [GPC, OUT_DIM], F32, isOutput=False)
    p_ones = nc.declare_dram_parameter("ones1", [1, 128], F32, isOutput=False)
    p_out = nc.declare_dram_parameter("out", [NPC, OUT_DIM], F32, isOutput=True)

    AG_RG = [list(range(NCORES))]
    calls = sched["calls"]
    NCK = sched["nckmax"]

    qctr = [0]

    with tile.TileContext(nc) as tc:
        with (
            tc.tile_pool(name="const", bufs=1) as const,
            tc.tile_pool(name="acts", bufs=2) as acts,
            tc.tile_pool(name="gbpA", bufs=LOOKA + 1) as gbpA,
            tc.tile_pool(name="gbpB", bufs=3) as gbpB,
            tc.tile_pool(name="stp", bufs=2) as stp,
            tc.tile_pool(name="work", bufs=2) as work,
            tc.tile_pool(name="stats", bufs=1) as stats,
            tc.tile_pool(name="pscat", bufs=3, space="PSUM") as pscat,
            tc.tile_pool(name="pmisc", bufs=2, space="PSUM") as pmisc,
            tc.tile_pool(name="dram", bufs=2, space="DRAM") as dram,
        ):
            nc.gpsimd.load_library(mlp_library)

            # ---- load constants ----
            def cload(p, shape, dtype, tag):
                t = const.tile(shape, dtype, tag=tag, name=tag)
                nc.sync.dma_start(t[:], p[:, :])
                return t

            idx_sb = cload(p_idx, [128, max(sched["idx_cols"], 8)], I16, "idx")
            dstrel_sb = cload(p_dstrel, list(p_dstrel.shape), BF16, "dstrel")
            iota_sb = cload(p_iota, [128, 256], BF16, "iota")
            wproj_sb = cload(p_wproj, [IN_DIM, HID], F32, "wproj")
            wl_sb = cload(p_wl, [HID, LAYERS * HID], F32, "wl")
            wa_sb = cload(p_wa, [HID, LAYERS * HID], F32, "wa")
            bag4_sb = cload(p_bag4, [1, LAYERS * 512], F32, "bag4")
            biaspb_sb = cload(p_biaspb, [128, 1 + LAYERS], F32, "biaspb")
            bn_sb = cload(p_bn, [128, 4], F32, "bn")
            wfx_sb = cload(p_wfx, [HID, OUT_DIM], F32, "wfx")
            wfu_sb = cload(p_wfu, [HID, OUT_DIM], F32, "wfu")
            bfin_sb = cload(p_bfin, [GPC, OUT_DIM], F32, "bfin")
            ones_sb = cload(p_ones, [1, 128], F32, "ones1")

            # iota256 replicated along the chunk axis for bulk S-builds
            iota_rep = const.tile([128, NCK + 1, 256], BF16, tag="iota_rep")
            for k in range(NCK + 1):
                nc.vector.tensor_copy(iota_rep[:, k, :], iota_sb[:])

            # zero the gather buffers once: trailing -1 slots are skipped by
            # desc-gen, so those partitions keep stale SBUF data (NaN shield).
            for _ in range(LOOKA + 1):
                t = gbpA.tile([128, NCK, HID], BF16, tag="gb0", name="gbz")
                nc.vector.memset(t[:], 0.0)
            for _ in range(3):
                t = gbpB.tile([128, NCK, HID], BF16, tag="gb1", name="gbz")
                nc.vector.memset(t[:], 0.0)

            def new_uloc(h):
                return dram.tile([HALFPC, HID], BF16, tag=f"uloc{h}",
                                 name=f"uloc{h}")

            def emit_uloc_group(xsrc, wa_i, li_target, h, t4, uloc):
                """One [128, 512] group of u_loc half h (tiles 4*t4..4*t4+3
                within the half) -> DRAM, with b_aggr folded in."""
                u3 = uloc.rearrange("(t p) h -> t p h", p=128)
                ps = pmisc.tile([128, 512], F32, tag="mm512", name="ps")
                ub = work.tile([128, 512], BF16, tag="ubounce", name="ub")
                # b_aggr folded in via K=1 ones-outer-product matmuls
                for q in range(4):
                    t = h * (TPC // 2) + 4 * t4 + q
                    co = li_target * 512 + q * 128
                    nc.tensor.matmul(ps[:, q * 128:(q + 1) * 128],
                                     ones_sb[:], bag4_sb[:, co:co + 128],
                                     start=True, stop=False)
                    nc.tensor.matmul(ps[:, q * 128:(q + 1) * 128],
                                     xsrc[:, t * 128:(t + 1) * 128], wa_i,
                                     start=False, stop=True)
                nc.scalar.activation(ub[:], ps[:],
                                     mybir.ActivationFunctionType.Copy)
                for q in range(4):
                    nc.sync.dma_start(u3[4 * t4 + q],
                                      ub[:, q * 128:(q + 1) * 128])

            def emit_ag(uloc, h, par):
                tbl = dram.tile([TROWS, HID], BF16, tag=f"table{h}",
                                name=f"table{h}")
                nc.gpsimd.collective_compute(
                    "AllGather", mybir.AluOpType.bypass,
                    replica_groups=AG_RG,
                    ins=[uloc[:].opt()],
                    outs=[tbl[:].opt()],
                )
                return tbl

            # ---- input projection + relu (x0 streamed in 512-col chunks);
            # layer-0 uloc groups interleave with proj; AG_A fires mid-proj,
            # AG_B right after proj (both tables ready before layer 0).
            xT = acts.tile([HID, NPC], F32, tag="x")
            uloc_next = [new_uloc(0), new_uloc(1)]
            tables_next = [None, None]
            for j in range(NPC // 512):
                x0c = work.tile([IN_DIM, 512], F32, tag="x0c")
                nc.sync.dma_start(x0c[:], p_xT0[:, j * 512:(j + 1) * 512])
                ps = pmisc.tile([128, 512], F32, tag="mm512", name="ps")
                nc.tensor.matmul(ps[:], wproj_sb[:], x0c[:],
                                 start=True, stop=True)
                nc.scalar.activation(xT[:, j * 512:(j + 1) * 512], ps[:],
                                     mybir.ActivationFunctionType.Relu,
                                     bias=biaspb_sb[:, 0:1])
                h, t4 = (0, j) if j < 8 else (1, j - 8)
                emit_uloc_group(xT, wa_sb[:, 0:HID], 0, h, t4, uloc_next[h])
                if j == 7:
                    tables_next[0] = emit_ag(uloc_next[0], 0, 0)
            tables_next[1] = emit_ag(uloc_next[1], 1, 0)

            ug_parts = stats.tile([128, NPP], F32, tag="ug_parts")
            sx_parts = stats.tile([128, NPP], F32, tag="sx_parts")
            ssx_parts = stats.tile([128, NPP], F32, tag="ssx_parts")
            scrap = stats.tile([128, 256], BF16, tag="scrap")

            # ---- message-passing layers ----
            for li in range(nlayers):
                wl_i = wl_sb[:, li * HID:(li + 1) * HID]
                wa_nx = wa_sb[:, (li + 1) * HID:(li + 2) * HID] if li + 1 < nlayers else None
                last = li == nlayers - 1

                tables = tables_next
                tables_next = [None, None]
                if not last:
                    uloc_next = [new_uloc(0), new_uloc(1)]

                xT_new = acts.tile([HID, NPC], F32, tag="x")

                def build_st(call):
                    # one-hot build: st[:, col, j] = (dstrel[:, col] == -j)
                    ncols = call["nck"] + 1
                    st = stp.tile([128, NCK + 1, 256], BF16,
                                  tag=f"st{call['half']}", name="st")
                    so = call["st_off"]
                    nc.vector.tensor_tensor(
                        st[:, 0:ncols, :], iota_rep[:, 0:ncols, :],
                        dstrel_sb[:, so:so + ncols, None].broadcast_to([128, ncols, 256]),
                        mybir.AluOpType.is_equal)
                    return st

                def gather(call, pool):
                    nck = call["nck"]
                    h = call["half"]
                    gb = pool.tile([128, NCK, HID], BF16, tag=f"gb{h}",
                                   name="gb")
                    nidx = nck * 128
                    nc.gpsimd.dma_gather(
                        gb[:, 0:nck, :], tables[h][:, :],
                        idx_sb[:, call["idx_col"]:call["idx_col"] + nck * 8],
                        nidx, nidx, HID, single_packet=False,
                        queue_num=qctr[0] % NQUEUES,
                    )
                    qctr[0] += 1
                    return gb

                # A-half gathers run LOOKA batches ahead.
                gbA = {}
                for j in range(min(LOOKA, NB)):
                    gbA[j] = gather(calls[2 * j], gbpA)

                for b in range(NB):
                    callA, callB = calls[2 * b], calls[2 * b + 1]
                    gbufs = {0: (gbA.pop(b), callA),
                             1: (gather(callB, gbpB), callB)}
                    sbufs = {0: build_st(callA), 1: build_st(callB)}

                    for p in (callA["p0"], callA["p1"]):
                        pt = pscat.tile([128, 256], F32, tag="scat")
                        # collect spans over both halves
                        todo = []
                        for h in (0, 1):
                            gb, call = gbufs[h]
                            st = sbufs[h]
                            spans = call["spans0"] if p == call["p0"] else call["spans1"]
                            for (ck, sc) in spans:
                                todo.append((gb, st, ck, sc))
                        for i, (gb, st, ck, sc) in enumerate(todo):
                            fin = last and i == len(todo) - 1
                            nc.tensor.matmul(pt[:], gb[:, ck, :],
                                             st[:, sc, :],
                                             start=(i == 0), stop=fin)
                        if not last:
                            # x_i = x @ W_layers accumulated on top
                            nc.tensor.matmul(pt[:], wl_i,
                                             xT[:, p * 256:(p + 1) * 256],
                                             start=False, stop=True)
                            nc.scalar.activation(
                                xT_new[:, p * 256:(p + 1) * 256], pt[:],
                                mybir.ActivationFunctionType.Relu,
                                bias=biaspb_sb[:, li + 1:li + 2])
                        else:
                            # u finished: per-pair u sums on ScalarE, then add
                            # x_i from a separate psum tile on DVE + relu.
                            nc.scalar.activation(
                                scrap[:], pt[:],
                                mybir.ActivationFunctionType.Copy,
                                accum_out=ug_parts[:, p:p + 1])
                            pxi = pmisc.tile([128, 256], F32, tag="mmfin")
                            nc.tensor.matmul(pxi[:], wl_i,
                                             xT[:, p * 256:(p + 1) * 256],
                                             start=True, stop=True)
                            xi_sb = work.tile([128, 256], F32, tag="xisb")
                            nc.scalar.activation(
                                xi_sb[:], pxi[:],
                                mybir.ActivationFunctionType.Copy)
                            tmp = work.tile([128, 256], F32, tag="xtmp")
                            nc.vector.scalar_tensor_tensor(
                                tmp[:], pt[:], biaspb_sb[:, li + 1:li + 2],
                                xi_sb[:], mybir.AluOpType.add,
                                mybir.AluOpType.add)
                            nc.vector.tensor_scalar(
                                xT_new[:, p * 256:(p + 1) * 256], tmp[:], 0.0,
                                None, mybir.AluOpType.max)
                            # BatchNorm sums inline (ScalarE)
                            nc.scalar.activation(
                                scrap[:], xT_new[:, p * 256:(p + 1) * 256],
                                mybir.ActivationFunctionType.Copy,
                                accum_out=sx_parts[:, p:p + 1])
                            nc.scalar.activation(
                                scrap[:], xT_new[:, p * 256:(p + 1) * 256],
                                mybir.ActivationFunctionType.Square,
                                accum_out=ssx_parts[:, p:p + 1])
                    nxt = b + LOOKA
                    if nxt < NB:
                        gbA[nxt] = gather(calls[2 * nxt], gbpA)
                    # next layer's uloc groups stream per batch as xT_new
                    # pairs complete; AG_A fires two batches after its half's
                    # stores complete so it is ready well before layer li+1.
                    if not last:
                        if b < NB // 2:
                            emit_uloc_group(xT_new, wa_nx, li + 1, 0, b,
                                            uloc_next[0])
                        else:
                            emit_uloc_group(xT_new, wa_nx, li + 1, 1,
                                            b - NB // 2, uloc_next[1])
                        if b == NB // 2 + 1:
                            tables_next[0] = emit_ag(uloc_next[0], 0,
                                                     (li + 1) % 2)
                # AG_B for the next layer fires at the end of this layer so
                # its latency hides under the next layer's A gathers.
                if not last:
                    tables_next[1] = emit_ag(uloc_next[1], 1, (li + 1) % 2)
                xT = xT_new

            # ---- BatchNorm statistics (sums already accumulated inline) ----
            ug = stats.tile([128, GPC], F32, tag="ug")
            for g in range(GPC):
                nc.vector.tensor_reduce(ug[:, g:g + 1],
                                        ug_parts[:, g * 4:(g + 1) * 4],
                                        mybir.AxisListType.X, mybir.AluOpType.add)
            ugsq = stats.tile([128, GPC], F32, tag="ugsq")
            nc.vector.scalar_tensor_tensor(ugsq[:], ug[:], 0.0, ug[:],
                                           mybir.AluOpType.bypass,
                                           mybir.AluOpType.mult)
            pack = stats.tile([128, 4], F32, tag="pack")
            nc.vector.tensor_reduce(pack[:, 0:1], sx_parts[:],
                                    mybir.AxisListType.X, mybir.AluOpType.add)
            nc.vector.tensor_reduce(pack[:, 1:2], ssx_parts[:],
                                    mybir.AxisListType.X, mybir.AluOpType.add)
            nc.vector.tensor_reduce(pack[:, 2:3], ug[:],
                                    mybir.AxisListType.X, mybir.AluOpType.add)
            nc.vector.tensor_reduce(pack[:, 3:4], ugsq[:],
                                    mybir.AxisListType.X, mybir.AluOpType.add)
            # scale u-channel partials by nodes-per-graph
            nc.vector.tensor_scalar_mul(pack[:, 2:3], pack[:, 2:3], float(NPG))
            nc.vector.tensor_scalar_mul(pack[:, 3:4], pack[:, 3:4], float(NPG))

            ar_in = dram.tile([128, 4], F32, tag="ar_in")
            ar_out = dram.tile([128, 4], F32, tag="ar_out")
            nc.sync.dma_start(ar_in[:], pack[:])
            nc.gpsimd.collective_compute(
                "AllReduce", mybir.AluOpType.add,
                replica_groups=AG_RG,
                ins=[ar_in[:].opt()],
                outs=[ar_out[:].opt()],
            )
            gstats = stats.tile([128, 4], F32, tag="gstats")
            nc.sync.dma_start(gstats[:], ar_out[:])

            # mean/var -> scale/bias per channel, for x-half and u-half
            sb = {}
            for half_i, (s_col, q_col, g_col, b_col) in enumerate(
                    [(0, 1, 0, 1), (2, 3, 2, 3)]):
                mean = stats.tile([128, 1], F32, tag=f"mean{half_i}")
                var = stats.tile([128, 1], F32, tag=f"var{half_i}")
                rstd = stats.tile([128, 1], F32, tag=f"rstd{half_i}")
                scl = stats.tile([128, 1], F32, tag=f"scl{half_i}")
                bia = stats.tile([128, 1], F32, tag=f"bia{half_i}")
                nc.vector.tensor_scalar_mul(mean[:], gstats[:, s_col:s_col + 1], 1.0 / N)
                nc.vector.tensor_scalar_mul(var[:], gstats[:, q_col:q_col + 1], 1.0 / N)
                tmp = stats.tile([128, 1], F32, tag=f"tmp{half_i}")
                nc.vector.scalar_tensor_tensor(tmp[:], mean[:], 0.0, mean[:],
                                               mybir.AluOpType.bypass,
                                               mybir.AluOpType.mult)
                nc.vector.scalar_tensor_tensor(var[:], var[:], 0.0, tmp[:],
                                               mybir.AluOpType.bypass,
                                               mybir.AluOpType.subtract)
                std = stats.tile([128, 1], F32, tag=f"std{half_i}")
                nc.vector.tensor_scalar_add(var[:], var[:], EPS)
                nc.scalar.activation(std[:], var[:],
                                     mybir.ActivationFunctionType.Sqrt)
                nc.vector.reciprocal(rstd[:], std[:])
                nc.vector.scalar_tensor_tensor(scl[:], rstd[:], 0.0,
                                               bn_sb[:, g_col:g_col + 1],
                                               mybir.AluOpType.bypass,
                                               mybir.AluOpType.mult)
                nc.vector.scalar_tensor_tensor(tmp[:], mean[:], 0.0, scl[:],
                                               mybir.AluOpType.bypass,
                                               mybir.AluOpType.mult)
                nc.vector.scalar_tensor_tensor(bia[:], bn_sb[:, b_col:b_col + 1],
                                               0.0, tmp[:],
                                               mybir.AluOpType.bypass,
                                               mybir.AluOpType.subtract)
                sb[half_i] = (scl, bia)

            # fold BN into the final matmul: out = xT @ (scl_x*wfx)
            #   + [ug @ (scl_u*wfu) + b_final + sum_ch(bia*W)] per graph
            wfxs = stats.tile([128, OUT_DIM], F32, tag="wfxs")
            nc.vector.tensor_scalar(wfxs[:], wfx_sb[:], sb[0][0][:], None,
                                    mybir.AluOpType.mult)
            wfus = stats.tile([128, OUT_DIM], F32, tag="wfus")
            nc.vector.tensor_scalar(wfus[:], wfu_sb[:], sb[1][0][:], None,
                                    mybir.AluOpType.mult)
            # bterm[1,2] = sum_ch bia_x*wfx + bia_u*wfu
            pb = pmisc.tile([1, OUT_DIM], F32, tag="mmfin")
            nc.tensor.matmul(pb[:], sb[0][1][:], wfx_sb[:], start=True, stop=False)
            nc.tensor.matmul(pb[:], sb[1][1][:], wfu_sb[:], start=False, stop=True)
            pb_sb = stats.tile([1, OUT_DIM], F32, tag="pb_sb")
            nc.scalar.activation(pb_sb[:], pb[:],
                                 mybir.ActivationFunctionType.Copy)

            # c_u[g,:] = ug[:,g] @ wfus + bterm (broadcast via K=1 matmul)
            cu_ps = pmisc.tile([GPC, OUT_DIM], F32, tag="mmfin")
            nc.tensor.matmul(cu_ps[:], ug[:], wfus[:], start=True, stop=False)
            nc.tensor.matmul(cu_ps[:], ones_sb[:, 0:GPC], pb_sb[:],
                             start=False, stop=True)
            cub = stats.tile([GPC, OUT_DIM], F32, tag="cub")
            nc.vector.scalar_tensor_tensor(cub[:], cu_ps[:], 0.0, bfin_sb[:],
                                           mybir.AluOpType.bypass,
                                           mybir.AluOpType.add)
            cub_dram = dram.tile([GPC, OUT_DIM], F32, tag="cub_dram")
            nc.sync.dma_start(cub_dram[:], cub[:])
            cubrow = stats.tile([1, GPC * OUT_DIM], F32, tag="cubrow")
            nc.sync.dma_start(
                cubrow[:], cub_dram[:].rearrange("g o -> (g o)")[None, :])

            # final matmul per tile + bias via K=1 trick (raw xT, scaled W);
            # results staged in SBUF, written out with a single DMA
            out_sb = stats.tile([128, TPC, OUT_DIM], F32, tag="out_sb")
            for t in range(TPC):
                g = t // 8
                psf = pmisc.tile([128, OUT_DIM], F32, tag="mmfin")
                nc.tensor.matmul(psf[:], xT[:, t * 128:(t + 1) * 128], wfxs[:],
                                 start=True, stop=False)
                nc.tensor.matmul(psf[:], ones_sb[:],
                                 cubrow[:, g * OUT_DIM:(g + 1) * OUT_DIM],
                                 start=False, stop=True)
                nc.vector.tensor_copy(out_sb[:, t, :], psf[:])
            nc.sync.dma_start(
                p_out[:, :].rearrange("(t p) o -> p t o", p=128), out_sb[:])

    nc.compile()
    return nc


def _bf16(a):
    return np.asarray(a, dtype=np.float32).astype(ml_dtypes.bfloat16)


def _make_in_maps(per_core, x, W_proj, b_proj, W_layers, b_layers, W_aggr,
                  b_aggr, bn_gamma, bn_beta, W_final, b_final):
    x = np.asarray(x, dtype=np.float32)
    iota_t = np.tile(-np.arange(256, dtype=np.float32), (128, 1))
    shared = dict(
        iota=_bf16(iota_t),
        wproj=np.asarray(W_proj, np.float32),
        wl=np.concatenate(list(np.asarray(W_layers, np.float32)), axis=1),
        wa=np.concatenate(list(np.asarray(W_aggr, np.float32)), axis=1),
        bag4=np.concatenate([np.tile(np.asarray(b_aggr, np.float32)[li], 4)
                             for li in range(LAYERS)]).reshape(1, LAYERS * 512),
        biaspb=np.concatenate(
            [np.asarray(b_proj, np.float32).reshape(128, 1),
             np.asarray(b_layers, np.float32).T], axis=1).astype(np.float32),
        bn=np.stack([np.asarray(bn_gamma, np.float32)[:128],
                     np.asarray(bn_beta, np.float32)[:128],
                     np.asarray(bn_gamma, np.float32)[128:],
                     np.asarray(bn_beta, np.float32)[128:]], axis=1).astype(np.float32),
        wfx=np.asarray(W_final, np.float32)[:HID],
        wfu=np.asarray(W_final, np.float32)[HID:],
        bfin=np.tile(np.asarray(b_final, np.float32).reshape(1, OUT_DIM),
                     (GPC, 1)).astype(np.float32),
        ones1=np.ones((1, 128), np.float32),
    )
    in_maps = []
    for c in range(NCORES):
        m = dict(shared)
        m["xT0"] = np.ascontiguousarray(x[c * NPC:(c + 1) * NPC].T)
        m["idx"] = per_core[c]["idx"]
        m["dstrel"] = per_core[c]["dstrel"]
        in_maps.append(m)
    return in_maps


def kernel(x, ei, n_nodes, W_proj, b_proj, W_layers, b_layers, W_aggr, b_aggr,
           bn_gamma, bn_beta, W_final, b_final):
    key = hash(np.asarray(ei).tobytes())
    if key not in _cache:
        sched, per_core = _host_prep(ei)
        nc = _build_nc(sched)
        _cache[key] = (nc, per_core)
    nc, per_core = _cache[key]
    in_maps = _make_in_maps(per_core, x, W_proj, b_proj, W_layers, b_layers,
                            W_aggr, b_aggr, bn_gamma, bn_beta, W_final, b_final)
    global _last_in_maps
    _last_in_maps = in_maps
    res = run_bass_kernel_spmd(nc, in_maps, core_ids=list(range(NCORES)))
    out = np.concatenate([res.results[c]["out"] for c in range(NCORES)], axis=0)
    return out.reshape(N // int(n_nodes), -1).astype(np.float32)


_last_in_maps = None


# revision 10
# speedup vs baseline: 1.6132x; 1.1349x over previous
"""GNN message-passing kernel for Trainium2 (8 NeuronCores, SPMD). v2

Strategy (hardcoded for the nn_DoormanAgent problem):
  - 65536 nodes = 64 graphs x 1024; shard 8192 nodes (8 graphs) per core.
  - Activations live transposed in SBUF: [128 HID partitions x nodes free], f32.
  - Per layer: u_loc = x @ W_aggr + b_aggr (bias folded into the table) ->
    DRAM in two halves; two AllGathers produce tableA/tableB
    ([32768,128] bf16 each, int16-indexable) into Shared scratchpad.
  - Edges grouped per call = (dst tile PAIR x 2, src half): slot stream is
    [p0 edges | gap pad(idx 0) | p1 edges | trailing -1]; trailing -1 idxs
    are skipped by the Q7 desc-gen (cost ~= actual edges, not padded).
    The boundary chunk is "mixed" and is matmul'd once per pair with
    separate one-hot columns.
  - gpsimd.dma_gather desc-gen is the bottleneck engine; calls rotate over
    4 SWDGE queues so desc-gen parallelizes across Q7 core pairs (~2.8x).
  - Segment-sum on TensorE via one-hot S [slots, 256] built in bulk on
    VectorE in bf16 (2x DVE rate); psum is [128, 256] per dst pair.
  - A-half gathers run ahead with a lookahead window; AG_B for layer i
    fires at the END of layer i-1 so B gathers never stall.
  - Final BatchNorm via per-channel partial sums + a tiny AllReduce; last
    matmul 256->2 done per 128-node tile with a K=1 bias trick.
"""

import sys

sys.path.insert(0, "/opt/trn_rl_repo")

import numpy as np
import ml_dtypes

import concourse.bass as bass
import concourse.bacc as bacc
import concourse.mybir as mybir
import concourse.tile as tile
from concourse.bass_utils import run_bass_kernel_spmd
from concourse.library_config import mlp as mlp_library

BF16 = mybir.dt.bfloat16
F32 = mybir.dt.float32
I16 = mybir.dt.int16

N = 65536
E = 524288
NCORES = 8
NPC = N // NCORES            # 8192 nodes per core
TPC = NPC // 128             # 64 dst tiles per core
NPP = TPC // 2               # 32 dst tile-pairs per core (256 nodes each)
NPG = 1024                   # nodes per graph
GPC = NPC // NPG             # 8 graphs per core
IN_DIM, HID, OUT_DIM, LAYERS = 64, 128, 2, 3
EPS = 1e-5
HALFPC = NPC // 2            # 4096: rows per core per table half
TROWS = HALFPC * NCORES      # 32768 rows per table (int16-safe)
NB = NPP                     # 32... overwritten below
CALL_PAIRS = 2               # tile pairs per gather call (512 dst nodes)
NB = NPP // CALL_PAIRS       # 16 batches
LOOKA = 7                    # A-half gather lookahead (batches)
NQUEUES = 4                  # SWDGE queues (desc-gen core-pair parallelism)
LOOKB = 2                    # B-half gather lookahead (batches)

_cache = {}


def _host_prep(ei):
    """Group each core's incident edges per call = (dst tile pair, src half).

    Call slot stream: [p0 edges | gap pad (idx 0) | p1 edges | trailing -1].
    Trailing -1 indices are skipped by Q7 desc-gen.  The chunk straddling
    the p0/p1 boundary ("mixed") gets two one-hot columns.  Chunk counts
    are cross-core maxima so the SPMD instruction stream is identical."""
    src = np.asarray(ei[0], dtype=np.int64)
    dst = np.asarray(ei[1], dtype=np.int64)

    owner = src // NPC
    off = src % NPC
    tab_of = (off >= HALFPC).astype(np.int64)
    row_of = owner * HALFPC + off - tab_of * HALFPC
    core_of = dst // NPC
    pair_of = (dst % NPC) // 256
    rel_of = dst % 256

    # groups[c][p][h] = (rows, rels)
    groups = [[[None, None] for _ in range(NPP)] for _ in range(NCORES)]
    for c in range(NCORES):
        mc = core_of == c
        for h in (0, 1):
            m = mc & (tab_of == h)
            p_arr = pair_of[m]
            s_arr = row_of[m]
            r_arr = rel_of[m]
            order = np.argsort(p_arr, kind="stable")
            p_arr, s_arr, r_arr = p_arr[order], s_arr[order], r_arr[order]
            bounds = np.searchsorted(p_arr, np.arange(NPP + 1))
            for p in range(NPP):
                lo, hi = bounds[p], bounds[p + 1]
                groups[c][p][h] = (s_arr[lo:hi], r_arr[lo:hi])

    calls = []
    idx_cols = 0
    st_cols = 0
    for b in range(NB):
        p0, p1 = CALL_PAIRS * b, CALL_PAIRS * b + 1
        for h in (0, 1):
            n0 = [len(groups[c][p0][h][0]) for c in range(NCORES)]
            n1 = [len(groups[c][p1][h][0]) for c in range(NCORES)]
            K0 = max(1, max(-(-n // 128) for n in n0))
            ends = [max(n0[c], (K0 - 1) * 128) + n1[c] for c in range(NCORES)]
            nck = max(K0, max(-(-e // 128) for e in ends))
            spans0 = [(k, k) for k in range(K0)]
            spans1 = [(K0 - 1, K0)] + [(k, k + 1) for k in range(K0, nck)]
            calls.append(dict(half=h, p0=p0, p1=p1, K0=K0, nck=nck,
                              idx_col=idx_cols, st_off=st_cols,
                              spans0=spans0, spans1=spans1))
            idx_cols += nck * 8
            st_cols += nck + 1

    nckmax = max(c["nck"] for c in calls)
    sched = dict(calls=calls, idx_cols=idx_cols, st_cols=st_cols,
                 nckmax=nckmax)

    per_core = []
    for c in range(NCORES):
        idx16 = np.zeros((16, max(idx_cols, 8)), dtype=np.int16)
        dstrel = np.full((128, max(st_cols, 1)), 1000.0, dtype=np.float32)
        for call in calls:
            h, p0, p1, K0, nck = (call["half"], call["p0"], call["p1"],
                                  call["K0"], call["nck"])
            s0, r0 = groups[c][p0][h]
            s1, r1 = groups[c][p1][h]
            n0, n1 = len(s0), len(s1)
            p1s = max(n0, (K0 - 1) * 128)
            stream = np.zeros(nck * 128, dtype=np.int16)
            stream[:n0] = s0.astype(np.int16)
            stream[p1s:p1s + n1] = s1.astype(np.int16)
            # wrap: stream pos j -> idx16[j % 16, idx_col + j // 16]
            base = call["idx_col"]
            idx16[:, base:base + nck * 8] = stream.reshape(nck * 8, 16).T
            # one-hot codes, negated; 1000 = no dst
            so = call["st_off"]
            codes = np.full((128, nck + 1), 1000.0, dtype=np.float32)
            for j, r in enumerate(r0):          # p0 edges: cols 0..K0-1
                codes[j % 128, j // 128] = -float(r)
            for j2, r in enumerate(r1):         # p1 edges
                j = p1s + j2
                ck = j // 128
                col = K0 if ck == K0 - 1 else ck + 1
                codes[j % 128, col] = -float(r)
            dstrel[:, so:so + nck + 1] = codes
        idx = np.tile(idx16, (8, 1))
        per_core.append(dict(
            idx=idx,
            dstrel=dstrel.astype(ml_dtypes.bfloat16),
        ))
    return sched, per_core


def _build_nc(sched, nlayers=LAYERS):
    nc = bacc.Bacc("TRN2", target_bir_lowering=False, debug=False,
                   num_swdge_queues=NQUEUES)

    # ---- dram parameters (inputs) ----
    p_xT0 = nc.declare_dram_parameter("xT0", [IN_DIM, NPC], F32, isOutput=False)
    p_idx = nc.declare_dram_parameter("idx", [128, max(sched["idx_cols"], 8)], I16, isOutput=False)
    p_dstrel = nc.declare_dram_parameter("dstrel", [128, max(sched["st_cols"], 1)], BF16, isOutput=False)
    p_iota = nc.declare_dram_parameter("iota", [128, 256], BF16, isOutput=False)
    p_wproj = nc.declare_dram_parameter("wproj", [IN_DIM, HID], F32, isOutput=False)
    p_wl = nc.declare_dram_parameter("wl", [HID, LAYERS * HID], F32, isOutput=False)
    p_wa = nc.declare_dram_parameter("wa", [HID, LAYERS * HID], F32, isOutput=False)
    p_bag4 = nc.declare_dram_parameter("bag4", [1, LAYERS * 512], F32, isOutput=False)
    p_biaspb = nc.declare_dram_parameter("biaspb", [128, 1 + LAYERS], F32, isOutput=False)
    p_bn = nc.declare_dram_parameter("bn", [128, 4], F32, isOutput=False)
    p_wfx = nc.declare_dram_parameter("wfx", [HID, OUT_DIM], F32, isOutput=False)
    p_wfu = nc.declare_dram_parameter("wfu", [HID, OUT_DIM], F32, isOutput=False)
    p_bfin = nc.declare_dram_parameter("bfin", [GPC, OUT_DIM], F32, isOutput=False)
    p_ones = nc.declare_dram_parameter("ones1", [1, 128], F32, isOutput=False)
    p_out = nc.declare_dram_parameter("out", [NPC, OUT_DIM], F32, isOutput=True)

    AG_RG = [list(range(NCORES))]
    calls = sched["calls"]
    NCK = sched["nckmax"]

    qctr = [0]

    with tile.TileContext(nc) as tc:
        with (
            tc.tile_pool(name="const", bufs=1) as const,
            tc.tile_pool(name="acts", bufs=2) as acts,
            tc.tile_pool(name="gbpA", bufs=LOOKA + 1) as gbpA,
            tc.tile_pool(name="gbpB", bufs=3) as gbpB,
            tc.tile_pool(name="stp", bufs=2) as stp,
            tc.tile_pool(name="work", bufs=2) as work,
            tc.tile_pool(name="stats", bufs=1) as stats,
            tc.tile_pool(name="pscat", bufs=3, space="PSUM") as pscat,
            tc.tile_pool(name="pmisc", bufs=2, space="PSUM") as pmisc,
            tc.tile_pool(name="dram", bufs=2, space="DRAM") as dram,
        ):
            nc.gpsimd.load_library(mlp_library)

            # ---- load constants ----
            def cload(p, shape, dtype, tag):
                t = const.tile(shape, dtype, tag=tag, name=tag)
                nc.sync.dma_start(t[:], p[:, :])
                return t

            idx_sb = cload(p_idx, [128, max(sched["idx_cols"], 8)], I16, "idx")
            dstrel_sb = cload(p_dstrel, list(p_dstrel.shape), BF16, "dstrel")
            iota_sb = cload(p_iota, [128, 256], BF16, "iota")
            wproj_sb = cload(p_wproj, [IN_DIM, HID], F32, "wproj")
            wl_sb = cload(p_wl, [HID, LAYERS * HID], F32, "wl")
            wa_sb = cload(p_wa, [HID, LAYERS * HID], F32, "wa")
            bag4_sb = cload(p_bag4, [1, LAYERS * 512], F32, "bag4")
            biaspb_sb = cload(p_biaspb, [128, 1 + LAYERS], F32, "biaspb")
            bn_sb = cload(p_bn, [128, 4], F32, "bn")
            wfx_sb = cload(p_wfx, [HID, OUT_DIM], F32, "wfx")
            wfu_sb = cload(p_wfu, [HID, OUT_DIM], F32, "wfu")
            bfin_sb = cload(p_bfin, [GPC, OUT_DIM], F32, "bfin")
            ones_sb = cload(p_ones, [1, 128], F32, "ones1")

            # iota256 replicated along the chunk axis for bulk S-builds
            iota_rep = const.tile([128, NCK + 1, 256], BF16, tag="iota_rep")
            for k in range(NCK + 1):
                nc.vector.tensor_copy(iota_rep[:, k, :], iota_sb[:])

            # zero the gather buffers once: trailing -1 slots are skipped by
            # desc-gen, so those partitions keep stale SBUF data (NaN shield).
            for _ in range(LOOKA + 1):
                t = gbpA.tile([128, NCK, HID], BF16, tag="gb0", name="gbz")
                nc.vector.memset(t[:], 0.0)
            for _ in range(3):
                t = gbpB.tile([128, NCK, HID], BF16, tag="gb1", name="gbz")
                nc.vector.memset(t[:], 0.0)

            def new_uloc(h):
                return dram.tile([HALFPC, HID], BF16, tag=f"uloc{h}",
                                 name=f"uloc{h}")

            def emit_uloc_group(xsrc, wa_i, li_target, h, t4, uloc):
                """One [128, 512] group of u_loc half h (tiles 4*t4..4*t4+3
                within the half) -> DRAM, with b_aggr folded in."""
                u3 = uloc.rearrange("(t p) h -> t p h", p=128)
                ps = pmisc.tile([128, 512], F32, tag="mm512", name="ps")
                ub = work.tile([128, 512], BF16, tag="ubounce", name="ub")
                # b_aggr folded in via K=1 ones-outer-product matmuls
                for q in range(4):
                    t = h * (TPC // 2) + 4 * t4 + q
                    co = li_target * 512 + q * 128
                    nc.tensor.matmul(ps[:, q * 128:(q + 1) * 128],
                                     ones_sb[:], bag4_sb[:, co:co + 128],
                                     start=True, stop=False)
                    nc.tensor.matmul(ps[:, q * 128:(q + 1) * 128],
                                     xsrc[:, t * 128:(t + 1) * 128], wa_i,
                                     start=False, stop=True)
                nc.scalar.activation(ub[:], ps[:],
                                     mybir.ActivationFunctionType.Copy)
                for q in range(4):
                    nc.sync.dma_start(u3[4 * t4 + q],
                                      ub[:, q * 128:(q + 1) * 128])

            def emit_ag(uloc, h, par):
                tbl = dram.tile([TROWS, HID], BF16, tag=f"table{h}",
                                name=f"table{h}")
                nc.gpsimd.collective_compute(
                    "AllGather", mybir.AluOpType.bypass,
                    replica_groups=AG_RG,
                    ins=[uloc[:].opt()],
                    outs=[tbl[:].opt()],
                )
                return tbl

            # ---- input projection + relu (x0 streamed in 512-col chunks);
            # layer-0 uloc groups interleave with proj; AG_A fires mid-proj,
            # AG_B right after proj (both tables ready before layer 0).
            xT = acts.tile([HID, NPC], F32, tag="x")
            uloc_next = [new_uloc(0), new_uloc(1)]
            tables_next = [None, None]
            for j in range(NPC // 512):
                x0c = work.tile([IN_DIM, 512], F32, tag="x0c")
                nc.sync.dma_start(x0c[:], p_xT0[:, j * 512:(j + 1) * 512])
                ps = pmisc.tile([128, 512], F32, tag="mm512", name="ps")
                nc.tensor.matmul(ps[:], wproj_sb[:], x0c[:],
                                 start=True, stop=True)
                nc.scalar.activation(xT[:, j * 512:(j + 1) * 512], ps[:],
                                     mybir.ActivationFunctionType.Relu,
                                     bias=biaspb_sb[:, 0:1])
                h, t4 = (0, j) if j < 8 else (1, j - 8)
                emit_uloc_group(xT, wa_sb[:, 0:HID], 0, h, t4, uloc_next[h])
                if j == 7:
                    tables_next[0] = emit_ag(uloc_next[0], 0, 0)
            tables_next[1] = emit_ag(uloc_next[1], 1, 0)

            ug_parts = stats.tile([128, NPP], F32, tag="ug_parts")
            sx_parts = stats.tile([128, NPP], F32, tag="sx_parts")
            ssx_parts = stats.tile([128, NPP], F32, tag="ssx_parts")
            scrap = stats.tile([128, 256], BF16, tag="scrap")

            # ---- message-passing layers ----
            for li in range(nlayers):
                wl_i = wl_sb[:, li * HID:(li + 1) * HID]
                wa_nx = wa_sb[:, (li + 1) * HID:(li + 2) * HID] if li + 1 < nlayers else None
                last = li == nlayers - 1

                tables = tables_next
                tables_next = [None, None]
                if not last:
                    uloc_next = [new_uloc(0), new_uloc(1)]

                xT_new = acts.tile([HID, NPC], F32, tag="x")

                def build_st(call):
                    # one-hot build: st[:, col, j] = (dstrel[:, col] == -j)
                    ncols = call["nck"] + 1
                    st = stp.tile([128, NCK + 1, 256], BF16,
                                  tag=f"st{call['half']}", name="st")
                    so = call["st_off"]
                    nc.vector.tensor_tensor(
                        st[:, 0:ncols, :], iota_rep[:, 0:ncols, :],
                        dstrel_sb[:, so:so + ncols, None].broadcast_to([128, ncols, 256]),
                        mybir.AluOpType.is_equal)
                    return st

                def gather(call, pool):
                    nck = call["nck"]
                    h = call["half"]
                    gb = pool.tile([128, NCK, HID], BF16, tag=f"gb{h}",
                                   name="gb")
                    nidx = nck * 128
                    nc.gpsimd.dma_gather(
                        gb[:, 0:nck, :], tables[h][:, :],
                        idx_sb[:, call["idx_col"]:call["idx_col"] + nck * 8],
                        nidx, nidx, HID, single_packet=False,
                        queue_num=qctr[0] % NQUEUES,
                    )
                    qctr[0] += 1
                    return gb

                # A gathers run LOOKA batches ahead, B gathers LOOKB,
                # one-hot builds one batch ahead: Pool/DVE stay fed while
                # TensorE consumes the previous batch.
                gbA = {}
                gbB = {}
                stc = {}
                for j in range(min(LOOKA, NB)):
                    gbA[j] = gather(calls[2 * j], gbpA)
                for j in range(min(LOOKB, NB)):
                    gbB[j] = gather(calls[2 * j + 1], gbpB)
                stc[0] = {0: build_st(calls[0]), 1: build_st(calls[1])}

                for b in range(NB):
                    callA, callB = calls[2 * b], calls[2 * b + 1]
                    if b + LOOKB < NB:
                        gbB[b + LOOKB] = gather(calls[2 * (b + LOOKB) + 1], gbpB)
                    if b + LOOKA < NB:
                        gbA[b + LOOKA] = gather(calls[2 * (b + LOOKA)], gbpA)
                    if b + 1 < NB:
                        stc[b + 1] = {0: build_st(calls[2 * b + 2]),
                                      1: build_st(calls[2 * b + 3])}
                    gbufs = {0: (gbA.pop(b), callA), 1: (gbB.pop(b), callB)}
                    sbufs = stc.pop(b)

                    for p in (callA["p0"], callA["p1"]):
                        pt = pscat.tile([128, 256], F32, tag="scat")
                        # collect spans over both halves
                        todo = []
                        for h in (0, 1):
                            gb, call = gbufs[h]
                            st = sbufs[h]
                            spans = call["spans0"] if p == call["p0"] else call["spans1"]
                            for (ck, sc) in spans:
                                todo.append((gb, st, ck, sc))
                        for i, (gb, st, ck, sc) in enumerate(todo):
                            fin = last and i == len(todo) - 1
                            nc.tensor.matmul(pt[:], gb[:, ck, :],
                                             st[:, sc, :],
                                             start=(i == 0), stop=fin)
                        if not last:
                            # x_i = x @ W_layers accumulated on top
                            nc.tensor.matmul(pt[:], wl_i,
                                             xT[:, p * 256:(p + 1) * 256],
                                             start=False, stop=True)
                            nc.scalar.activation(
                                xT_new[:, p * 256:(p + 1) * 256], pt[:],
                                mybir.ActivationFunctionType.Relu,
                                bias=biaspb_sb[:, li + 1:li + 2])
                        else:
                            # u finished: per-pair u sums on ScalarE, then add
                            # x_i from a separate psum tile on DVE + relu.
                            nc.scalar.activation(
                                scrap[:], pt[:],
                                mybir.ActivationFunctionType.Copy,
                                accum_out=ug_parts[:, p:p + 1])
                            pxi = pmisc.tile([128, 256], F32, tag="mmfin")
                            nc.tensor.matmul(pxi[:], wl_i,
                                             xT[:, p * 256:(p + 1) * 256],
                                             start=True, stop=True)
                            xi_sb = work.tile([128, 256], F32, tag="xisb")
                            nc.scalar.activation(
                                xi_sb[:], pxi[:],
                                mybir.ActivationFunctionType.Copy)
                            tmp = work.tile([128, 256], F32, tag="xtmp")
                            nc.vector.scalar_tensor_tensor(
                                tmp[:], pt[:], biaspb_sb[:, li + 1:li + 2],
                                xi_sb[:], mybir.AluOpType.add,
                                mybir.AluOpType.add)
                            nc.scalar.activation(
                                xT_new[:, p * 256:(p + 1) * 256], tmp[:],
                                mybir.ActivationFunctionType.Relu)
                            # BatchNorm sums inline (ScalarE)
                            nc.scalar.activation(
                                scrap[:], xT_new[:, p * 256:(p + 1) * 256],
                                mybir.ActivationFunctionType.Copy,
                                accum_out=sx_parts[:, p:p + 1])
                            nc.scalar.activation(
                                scrap[:], xT_new[:, p * 256:(p + 1) * 256],
                                mybir.ActivationFunctionType.Square,
                                accum_out=ssx_parts[:, p:p + 1])
                    # next layer's uloc groups stream per batch as xT_new
                    # pairs complete; AG_A fires two batches after its half's
                    # stores complete so it is ready well before layer li+1.
                    if not last:
                        if b < NB // 2:
                            emit_uloc_group(xT_new, wa_nx, li + 1, 0, b,
                                            uloc_next[0])
                        else:
                            emit_uloc_group(xT_new, wa_nx, li + 1, 1,
                                            b - NB // 2, uloc_next[1])
                        if b == NB // 2 + 1:
                            tables_next[0] = emit_ag(uloc_next[0], 0,
                                                     (li + 1) % 2)
                # AG_B for the next layer fires at the end of this layer so
                # its latency hides under the next layer's A gathers.
                if not last:
                    tables_next[1] = emit_ag(uloc_next[1], 1, (li + 1) % 2)
                xT = xT_new

            # ---- BatchNorm statistics (sums already accumulated inline) ----
            ug = stats.tile([128, GPC], F32, tag="ug")
            for g in range(GPC):
                nc.vector.tensor_reduce(ug[:, g:g + 1],
                                        ug_parts[:, g * 4:(g + 1) * 4],
                                        mybir.AxisListType.X, mybir.AluOpType.add)
            ugsq = stats.tile([128, GPC], F32, tag="ugsq")
            nc.vector.scalar_tensor_tensor(ugsq[:], ug[:], 0.0, ug[:],
                                           mybir.AluOpType.bypass,
                                           mybir.AluOpType.mult)
            pack = stats.tile([128, 4], F32, tag="pack")
            nc.vector.tensor_reduce(pack[:, 0:1], sx_parts[:],
                                    mybir.AxisListType.X, mybir.AluOpType.add)
            nc.vector.tensor_reduce(pack[:, 1:2], ssx_parts[:],
                                    mybir.AxisListType.X, mybir.AluOpType.add)
            nc.vector.tensor_reduce(pack[:, 2:3], ug[:],
                                    mybir.AxisListType.X, mybir.AluOpType.add)
            nc.vector.tensor_reduce(pack[:, 3:4], ugsq[:],
                                    mybir.AxisListType.X, mybir.AluOpType.add)
            # scale u-channel partials by nodes-per-graph
            nc.vector.tensor_scalar_mul(pack[:, 2:3], pack[:, 2:3], float(NPG))
            nc.vector.tensor_scalar_mul(pack[:, 3:4], pack[:, 3:4], float(NPG))

            ar_in = dram.tile([128, 4], F32, tag="ar_in")
            ar_out = dram.tile([128, 4], F32, tag="ar_out")
            nc.sync.dma_start(ar_in[:], pack[:])
            nc.gpsimd.collective_compute(
                "AllReduce", mybir.AluOpType.add,
                replica_groups=AG_RG,
                ins=[ar_in[:].opt()],
                outs=[ar_out[:].opt()],
            )
            gstats = stats.tile([128, 4], F32, tag="gstats")
            nc.sync.dma_start(gstats[:], ar_out[:])

            # mean/var -> scale/bias per channel, for x-half and u-half
            sb = {}
            for half_i, (s_col, q_col, g_col, b_col) in enumerate(
                    [(0, 1, 0, 1), (2, 3, 2, 3)]):
                mean = stats.tile([128, 1], F32, tag=f"mean{half_i}")
                var = stats.tile([128, 1], F32, tag=f"var{half_i}")
                rstd = stats.tile([128, 1], F32, tag=f"rstd{half_i}")
                scl = stats.tile([128, 1], F32, tag=f"scl{half_i}")
                bia = stats.tile([128, 1], F32, tag=f"bia{half_i}")
                nc.vector.tensor_scalar_mul(mean[:], gstats[:, s_col:s_col + 1], 1.0 / N)
                nc.vector.tensor_scalar_mul(var[:], gstats[:, q_col:q_col + 1], 1.0 / N)
                tmp = stats.tile([128, 1], F32, tag=f"tmp{half_i}")
                nc.vector.scalar_tensor_tensor(tmp[:], mean[:], 0.0, mean[:],
                                               mybir.AluOpType.bypass,
                                               mybir.AluOpType.mult)
                nc.vector.scalar_tensor_tensor(var[:], var[:], 0.0, tmp[:],
                                               mybir.AluOpType.bypass,
                                               mybir.AluOpType.subtract)
                std = stats.tile([128, 1], F32, tag=f"std{half_i}")
                nc.vector.tensor_scalar_add(var[:], var[:], EPS)
                nc.scalar.activation(std[:], var[:],
                                     mybir.ActivationFunctionType.Sqrt)
                nc.vector.reciprocal(rstd[:], std[:])
                nc.vector.scalar_tensor_tensor(scl[:], rstd[:], 0.0,
                                               bn_sb[:, g_col:g_col + 1],
                                               mybir.AluOpType.bypass,
                                               mybir.AluOpType.mult)
                nc.vector.scalar_tensor_tensor(tmp[:], mean[:], 0.0, scl[:],
                                               mybir.AluOpType.bypass,
                                               mybir.AluOpType.mult)
                nc.vector.scalar_tensor_tensor(bia[:], bn_sb[:, b_col:b_col + 1],
                                               0.0, tmp[:],
                                               mybir.AluOpType.bypass,
                                               mybir.AluOpType.subtract)
                sb[half_i] = (scl, bia)

            # fold BN into the final matmul: out = xT @ (scl_x*wfx)
            #   + [ug @ (scl_u*wfu) + b_final + sum_ch(bia*W)] per graph
            wfxs = stats.tile([128, OUT_DIM], F32, tag="wfxs")
            nc.vector.tensor_scalar(wfxs[:], wfx_sb[:], sb[0][0][:], None,
                                    mybir.AluOpType.mult)
            wfus = stats.tile([128, OUT_DIM], F32, tag="wfus")
            nc.vector.tensor_scalar(wfus[:], wfu_sb[:], sb[1][0][:], None,
                                    mybir.AluOpType.mult)
            # bterm[1,2] = sum_ch bia_x*wfx + bia_u*wfu
            pb = pmisc.tile([1, OUT_DIM], F32, tag="mmfin")
            nc.tensor.matmul(pb[:], sb[0][1][:], wfx_sb[:], start=True, stop=False)
            nc.tensor.matmul(pb[:], sb[1][1][:], wfu_sb[:], start=False, stop=True)
            pb_sb = stats.tile([1, OUT_DIM], F32, tag="pb_sb")
            nc.scalar.activation(pb_sb[:], pb[:],
                                 mybir.ActivationFunctionType.Copy)

            # c_u[g,:] = ug[:,g] @ wfus + bterm (broadcast via K=1 matmul)
            cu_ps = pmisc.tile([GPC, OUT_DIM], F32, tag="mmfin")
            nc.tensor.matmul(cu_ps[:], ug[:], wfus[:], start=True, stop=False)
            nc.tensor.matmul(cu_ps[:], ones_sb[:, 0:GPC], pb_sb[:],
                             start=False, stop=True)
            cub = stats.tile([GPC, OUT_DIM], F32, tag="cub")
            nc.vector.scalar_tensor_tensor(cub[:], cu_ps[:], 0.0, bfin_sb[:],
                                           mybir.AluOpType.bypass,
                                           mybir.AluOpType.add)
            cub_dram = dram.tile([GPC, OUT_DIM], F32, tag="cub_dram")
            nc.sync.dma_start(cub_dram[:], cub[:])
            cubrow = stats.tile([1, GPC * OUT_DIM], F32, tag="cubrow")
            nc.sync.dma_start(
                cubrow[:], cub_dram[:].rearrange("g o -> (g o)")[None, :])

            # final matmul per tile + bias via K=1 trick (raw xT, scaled W);
            # results staged in SBUF, written out with a single DMA
            out_sb = stats.tile([128, TPC, OUT_DIM], F32, tag="out_sb")
            for t in range(TPC):
                g = t // 8
                psf = pmisc.tile([128, OUT_DIM], F32, tag="mmfin")
                nc.tensor.matmul(psf[:], xT[:, t * 128:(t + 1) * 128], wfxs[:],
                                 start=True, stop=False)
                nc.tensor.matmul(psf[:], ones_sb[:],
                                 cubrow[:, g * OUT_DIM:(g + 1) * OUT_DIM],
                                 start=False, stop=True)
                nc.vector.tensor_copy(out_sb[:, t, :], psf[:])
            nc.sync.dma_start(
                p_out[:, :].rearrange("(t p) o -> p t o", p=128), out_sb[:])

    nc.compile()
    return nc


def _bf16(a):
    return np.asarray(a, dtype=np.float32).astype(ml_dtypes.bfloat16)


def _make_in_maps(per_core, x, W_proj, b_proj, W_layers, b_layers, W_aggr,
                  b_aggr, bn_gamma, bn_beta, W_final, b_final):
    x = np.asarray(x, dtype=np.float32)
    iota_t = np.tile(-np.arange(256, dtype=np.float32), (128, 1))
    shared = dict(
        iota=_bf16(iota_t),
        wproj=np.asarray(W_proj, np.float32),
        wl=np.concatenate(list(np.asarray(W_layers, np.float32)), axis=1),
        wa=np.concatenate(list(np.asarray(W_aggr, np.float32)), axis=1),
        bag4=np.concatenate([np.tile(np.asarray(b_aggr, np.float32)[li], 4)
                             for li in range(LAYERS)]).reshape(1, LAYERS * 512),
        biaspb=np.concatenate(
            [np.asarray(b_proj, np.float32).reshape(128, 1),
             np.asarray(b_layers, np.float32).T], axis=1).astype(np.float32),
        bn=np.stack([np.asarray(bn_gamma, np.float32)[:128],
                     np.asarray(bn_beta, np.float32)[:128],
                     np.asarray(bn_gamma, np.float32)[128:],
                     np.asarray(bn_beta, np.float32)[128:]], axis=1).astype(np.float32),
        wfx=np.asarray(W_final, np.float32)[:HID],
        wfu=np.asarray(W_final, np.float32)[HID:],
        bfin=np.tile(np.asarray(b_final, np.float32).reshape(1, OUT_DIM),
                     (GPC, 1)).astype(np.float32),
        ones1=np.ones((1, 128), np.float32),
    )
    in_maps = []
    for c in range(NCORES):
        m = dict(shared)
        m["xT0"] = np.ascontiguousarray(x[c * NPC:(c + 1) * NPC].T)
        m["idx"] = per_core[c]["idx"]
        m["dstrel"] = per_core[c]["dstrel"]
        in_maps.append(m)
    return in_maps


def kernel(x, ei, n_nodes, W_proj, b_proj, W_layers, b_layers, W_aggr, b_aggr,
           bn_gamma, bn_beta, W_final, b_final):
    key = hash(np.asarray(ei).tobytes())
    if key not in _cache:
        sched, per_core = _host_prep(ei)
        nc = _build_nc(sched)
        _cache[key] = (nc, per_core)
    nc, per_core = _cache[key]
    in_maps = _make_in_maps(per_core, x, W_proj, b_proj, W_layers, b_layers,
                            W_aggr, b_aggr, bn_gamma, bn_beta, W_final, b_final)
    global _last_in_maps
    _last_in_maps = in_maps
    res = run_bass_kernel_spmd(nc, in_maps, core_ids=list(range(NCORES)))
    out = np.concatenate([res.results[c]["out"] for c in range(NCORES)], axis=0)
    return out.reshape(N // int(n_nodes), -1).astype(np.float32)


_last_in_maps = None
